# revision 1
# baseline (speedup 1.0000x reference)
"""Trainium2 Bass kernel for nn_MultiHeadAttention (B=2, S=2048, D=1024, H=16).

Sharding: 8 cores = data-parallel over batch (2) x tensor-parallel over heads
(4 groups of 4 heads).  Each core:
  - computes Q^T, K^T (transposed [channels, seq] layout) and V (natural
    [seq, channels] layout, augmented with a ones column per head) for its
    4 heads via bf16 matmuls,
  - runs causal flash attention with *transposed* logits [k, q] so no
    transposes are needed anywhere (softmax sum over k comes out of the
    ones column of V as row 64 of the attention-output PSUM tile),
  - multiplies by a row-sharded slice of Wo, producing a partial [D, S]
    output in f32.
Host side: shards/pre-transposes inputs, sums the 4 partial Wo products per
batch entry (the tensor-parallel reduction) and adds the output bias.
"""

import sys

for _p in ("/opt/trn_rl_repo", "/root/.axon_site/_ro/trn_rl_repo"):
    if _p not in sys.path:
        sys.path.insert(0, _p)

import numpy as np
import ml_dtypes

import concourse.bass as bass  # noqa: F401  (registers engines)
import concourse.mybir as mybir
import concourse.tile as tile
from concourse import bacc
from concourse.bass_utils import run_bass_kernel_spmd

BF16 = ml_dtypes.bfloat16
F32 = np.float32

B, S, D, H, HD = 2, 2048, 1024, 16, 64
NCORES = 8
GROUPS = NCORES // B        # 4 head groups
HPG = H // GROUPS           # 4 heads per core
DCH = HPG * HD              # 256 channels per core
QT = 512                    # query tile width (free dim)
KC = 128                    # key chunk (partition dim)
NQT, NKC = S // QT, S // KC  # 4, 16
SCALE = 1.0 / 8.0           # 1/sqrt(HD)
NEG_BIG = -1.0e9

_NC_CACHE: dict = {}


def _classify(mask: np.ndarray):
    """Classify each (qtile, kchunk) block of the additive attention mask.

    Returns tuple-of-tuples of (kind, off) with kind in
    {skip, full, diag, gen}; off is the first visible column for diag blocks.
    """
    classes = []
    for qt in range(NQT):
        q0 = qt * QT
        row = []
        for kc in range(NKC):
            k0 = kc * KC
            blk = mask[q0:q0 + QT, k0:k0 + KC]
            if np.all(blk <= -1e8):
                row.append(("skip", 0))
            elif not blk.any():
                row.append(("full", 0))
            else:
                off = k0 - q0
                if 0 <= off < QT:
                    qi = np.arange(q0, q0 + QT)[:, None]
                    ki = np.arange(k0, k0 + KC)[None, :]
                    vis = qi >= ki
                    if (not blk[vis].any()) and np.all(blk[~vis] <= -1e8):
                        row.append(("diag", off))
                        continue
                row.append(("gen", 0))
        classes.append(tuple(row))

    has_gen = any(c[0] == "gen" for r in classes for c in r)
    if has_gen:
        # keep things simple/correct for odd masks: every non-skip block
        # takes the general (full-width + mask add) path
        classes = [
            tuple(("gen", 0) if c[0] in ("diag", "full") else c for c in r)
            for r in classes
        ]
    # first visible chunk of each qtile must cover the full tile width so the
    # accumulating matmul's start=True pass initializes every column
    for r in classes:
        first = next((c for c in r if c[0] != "skip"), None)
        assert first is None or first[1] == 0, "unsupported mask pattern"
    return tuple(tuple(r) for r in classes), has_gen


def _build(classes, has_gen: bool, skip_bias: bool = True, debug: bool = False):
    f32, bf16 = mybir.dt.float32, mybir.dt.bfloat16
    FT = mybir.ActivationFunctionType

    nc = bacc.Bacc("TRN2", target_bir_lowering=False, debug=False)

    # x inputs arrive pre-transposed + bf16, chunked into 4 seq tiles of 512
    xq_d = nc.dram_tensor("xq", [NQT, 128, 8, QT], bf16, kind="ExternalInput")
    xk_d = nc.dram_tensor("xk", [NQT, 128, 8, QT], bf16, kind="ExternalInput")
    xv_d = nc.dram_tensor("xv", [NQT, 128, 8, QT], bf16, kind="ExternalInput")
    wq_d = nc.dram_tensor("wq", [128, 8, DCH], bf16, kind="ExternalInput")
    wk_d = nc.dram_tensor("wk", [128, 8, DCH], bf16, kind="ExternalInput")
    wv_d = nc.dram_tensor("wv", [128, 8, DCH], bf16, kind="ExternalInput")
    wo_d = nc.dram_tensor("wo", [128, 2, D], bf16, kind="ExternalInput")
    bq_d = nc.dram_tensor("bq", [1, DCH], bf16, kind="ExternalInput")
    bk_d = nc.dram_tensor("bk", [1, DCH], bf16, kind="ExternalInput")
    bv_d = nc.dram_tensor("bv", [1, DCH], bf16, kind="ExternalInput")
    pad_d = nc.dram_tensor("pad", [128, NKC], f32, kind="ExternalInput")
    tril_d = nc.dram_tensor("tril", [128, 128], bf16, kind="ExternalInput")
    ident_d = nc.dram_tensor("ident", [128, 128], bf16, kind="ExternalInput")
    maskT_d = None
    if has_gen:
        maskT_d = nc.dram_tensor("maskT", [S, S], f32, kind="ExternalInput")
    out_d = nc.dram_tensor("out", [D, S], f32, kind="ExternalOutput")
    if debug:
        dbg_av = nc.dram_tensor("d_av", [2, 65, 512], f32, kind="ExternalOutput")
        dbg_at = nc.dram_tensor("d_at", [4, 128, 1024], f32, kind="ExternalOutput")
        dbg_r = nc.dram_tensor("d_r", [2, 1, 512], f32, kind="ExternalOutput")
        dbg_rb = nc.dram_tensor("d_rb", [2, HD, 512], f32, kind="ExternalOutput")

    with tile.TileContext(nc) as tc:
        with (
            tc.tile_pool(name="cpool", bufs=1) as cpool,
            tc.tile_pool(name="spool", bufs=(3 if debug else 5)) as spool,
            tc.tile_pool(name="apool", bufs=(6 if debug else 10)) as apool,
            tc.tile_pool(name="psA", bufs=2, space="PSUM") as psA,
            tc.tile_pool(name="psQK", bufs=2, space="PSUM") as psQK,
            tc.tile_pool(name="psAV", bufs=2, space="PSUM") as psAV,
        ):
            # ---------- constants ----------
            ones = cpool.tile([1, 512], bf16)
            nc.gpsimd.memset(ones[:], 1.0)
            tril_sb = cpool.tile([128, 128], bf16)
            nc.gpsimd.dma_start(tril_sb[:], tril_d[:])
            ident_sb = cpool.tile([128, 128], bf16)
            nc.gpsimd.dma_start(ident_sb[:], ident_d[:])
            pad_sb = cpool.tile([128, NKC], f32)
            nc.gpsimd.dma_start(pad_sb[:], pad_d[:])
            bq_sb = cpool.tile([1, DCH], bf16)
            nc.gpsimd.dma_start(bq_sb[:], bq_d[:])
            bk_sb = cpool.tile([1, DCH], bf16)
            nc.gpsimd.dma_start(bk_sb[:], bk_d[:])
            bv_sb = cpool.tile([1, DCH], bf16)
            nc.gpsimd.dma_start(bv_sb[:], bv_d[:])

            # ---------- weights / activations (x in 4 seq chunks) ----------
            def make_x(name):
                return [
                    cpool.tile([128, 8, QT], bf16, name=f"{name}{s4}")
                    for s4 in range(NQT)
                ]

            xk_sb, xq_sb, xv_sb = make_x("xk"), make_x("xq"), make_x("xv")
            # interleave weight + input DMAs so each pipeline stage can
            # start as soon as its seq-chunk lands
            wk_sb = cpool.tile([128, 8, DCH], bf16)
            nc.sync.dma_start(wk_sb[:], wk_d[:])
            nc.sync.dma_start(xk_sb[0][:], xk_d[0])
            wq_sb = cpool.tile([128, 8, DCH], bf16)
            nc.sync.dma_start(wq_sb[:], wq_d[:])
            nc.sync.dma_start(xq_sb[0][:], xq_d[0])
            wv_sb = cpool.tile([128, 8, DCH], bf16)
            nc.sync.dma_start(wv_sb[:], wv_d[:])
            nc.sync.dma_start(xv_sb[0][:], xv_d[0])
            for s4 in range(1, NQT):
                nc.sync.dma_start(xk_sb[s4][:], xk_d[s4])
                nc.sync.dma_start(xq_sb[s4][:], xq_d[s4])
                nc.sync.dma_start(xv_sb[s4][:], xv_d[s4])
            wo_sb = cpool.tile([128, 2, D], bf16)
            nc.sync.dma_start(wo_sb[:], wo_d[:])

            qT_sb = cpool.tile([128, 2, S], bf16)   # [pair-stacked ch, pair, s]
            kT_sb = cpool.tile([128, 2, S], bf16)
            v_sb = cpool.tile([128, NKC, HPG, HD + 1], bf16)  # ones col at 64
            nc.gpsimd.memset(v_sb[:], 1.0)  # preset ones columns
            ctxT_sb = cpool.tile([128, 2, S], bf16)

            # ---------- projections (one PSUM group per call) ----------
            def proj_qk_g(w_sb, b_sb, x_sb, dst, m, s4):
                ps = psA.tile([128, 512], f32, tag="proj",
                              name=f"pp{id(w_sb) % 97}_{m}{s4}")
                for kcc in range(8):
                    nc.tensor.matmul(
                        ps[:],
                        w_sb[:, kcc, m * 128:(m + 1) * 128],
                        x_sb[s4][:, kcc, :],
                        start=(kcc == 0),
                        stop=(kcc == 7 and skip_bias),
                    )
                if not skip_bias:
                    nc.tensor.matmul(
                        ps[:],
                        b_sb[0:1, m * 128:(m + 1) * 128],
                        ones[0:1, 0:512],
                        start=False, stop=True,
                    )
                nc.scalar.copy(
                    dst[:, m, s4 * 512:(s4 + 1) * 512], ps[:]
                )

            def proj_v_g(st):
                ps = psA.tile([128, 512], f32, tag="proj", name=f"pv{st}")
                pv = ps[:, 0:DCH]
                xt = xv_sb[st // 4][:, :, (st % 4) * 128:(st % 4 + 1) * 128]
                for kcc in range(8):
                    nc.tensor.matmul(
                        pv,
                        xt[:, kcc, :],
                        wv_sb[:, kcc, :],
                        start=(kcc == 0),
                        stop=(kcc == 7 and skip_bias),
                    )
                if not skip_bias:
                    nc.tensor.matmul(
                        pv, ones[0:1, 0:128], bv_sb[0:1, :],
                        start=False, stop=True,
                    )
                # NB: ACT mis-executes this strided 3D copy; keep it on DVE
                nc.vector.tensor_copy(
                    v_sb[:, st, :, 0:HD],
                    ps[:, 0:DCH].rearrange("p (h d) -> p h d", h=HPG),
                )

            # ---------- attention for one (qtile, pair) ----------
            def attn(qt, p_, filler=None):
                q0 = qt * QT
                chunks = [
                    (kc, classes[qt][kc])
                    for kc in range(NKC)
                    if classes[qt][kc][0] != "skip"
                ]
                av = [
                    psAV.tile([128, 512], f32, tag="av", name=f"av{qt}_{p_}_{i}")
                    for i in range(2)
                ]
                nchunks = len(chunks)

                def emit_av(idx, off, kc, att):
                    for i in range(2):
                        h_loc = 2 * p_ + i
                        nc.tensor.matmul(
                            av[i][0:HD + 1, off:QT],
                            v_sb[:, kc, h_loc, :],
                            att[:, i, off:QT],
                            start=(idx == 0), stop=(idx == nchunks - 1),
                        )

                pend = None
                for idx, (kc, (cls, off)) in enumerate(chunks):
                    pairps = psQK.tile([128, 1024], f32, tag="qk",
                                       name=f"qk{qt}_{p_}_{kc}")
                    pq = pairps.rearrange("p (h q) -> p h q", h=2)
                    dg = cls == "diag"
                    for i in range(2):
                        lo = 64 * i
                        nc.tensor.matmul(
                            pq[:, i, off:QT],
                            kT_sb[lo:lo + 64, p_, kc * KC:(kc + 1) * KC],
                            qT_sb[lo:lo + 64, p_, q0 + off:q0 + QT],
                            start=True, stop=not dg,
                        )
                    if dg:
                        # add the causal boundary mask on the PE itself:
                        # pq[:, i, off:off+128] += I.T @ trilT
                        for i in range(2):
                            nc.tensor.matmul(
                                pq[:, i, off:off + 128],
                                ident_sb[:], tril_sb[:],
                                start=False, stop=True,
                            )
                    elif cls == "gen":
                        mt = spool.tile([128, QT], f32, tag="genmask")
                        nc.sync.dma_start(
                            mt[:], maskT_d[kc * KC:(kc + 1) * KC, q0:q0 + QT]
                        )
                        nc.vector.tensor_add(
                            pq[:, :, :], pq[:, :, :],
                            mt[:, None, :].to_broadcast((128, 2, QT)),
                        )
                    at = apool.tile([128, 1024], bf16, tag="attnT")
                    att = at.rearrange("p (h q) -> p h q", h=2)
                    nc.scalar.activation(
                        att[:, :, off:], pq[:, :, off:], FT.Exp,
                        bias=pad_sb[:, kc:kc + 1], scale=SCALE,
                    )
                    if debug and qt == 0 and p_ == 0:
                        datc = spool.tile([128, 1024], f32, tag="dbgat")
                        nc.gpsimd.memset(datc[:], 0.0)
                        d3 = datc.rearrange("p (h q) -> p h q", h=2)
                        nc.vector.tensor_copy(d3[:, :, off:], att[:, :, off:])
                        nc.sync.dma_start(dbg_at[idx], datc[:])
                    # weave one group of independent PE work between the exp
                    # and its AV consumers so the PE stream has ready work
                    # while the activation engine computes the exp
                    if filler:
                        filler.popleft()()
                    # software-pipeline by one chunk: the AV matmuls for the
                    # PREVIOUS chunk are emitted after this chunk's QK/exp,
                    # so the PE never stalls on the in-flight exp
                    if pend is not None:
                        emit_av(*pend)
                    pend = (idx, off, kc, att)
                if pend is not None:
                    emit_av(*pend)
                if debug and qt == 0 and p_ == 0:
                    for i in range(2):
                        davc = spool.tile([65, 512], f32, tag="dbgav")
                        nc.vector.tensor_copy(davc[:], av[i][0:65, :])
                        nc.sync.dma_start(dbg_av[i], davc[:])
                # normalize: ctxT[h, q] = av[0:64, q] / av[64, q]
                for i in range(2):
                    # the denominator row is staged to a partition-0 SBUF
                    # tile: the custom-DVE approx reciprocal misreads PSUM
                    # and nonzero-base-partition sources on HW
                    dn = spool.tile([1, 512], f32, tag="denom")
                    nc.vector.tensor_copy(dn[:], av[i][HD:HD + 1, :])
                    r = spool.tile([1, 512], f32, tag="recip")
                    nc.vector.reciprocal_approx_fast(out=r[:], in_=dn[:])
                    dst = ctxT_sb[64 * i:64 * (i + 1), p_, q0:q0 + QT]
                    rb = spool.tile([HD, 512], f32, tag="rbcast")
                    nc.gpsimd.partition_broadcast(rb[:], r[:])
                    if debug and qt == 0 and p_ == 0:
                        nc.sync.dma_start(dbg_r[i], r[:])
                        nc.sync.dma_start(dbg_rb[i], rb[:])
                    nc.vector.tensor_mul(dst, av[i][0:HD, :], rb[:])

            def wo_g(qt, m, tail=False):
                q0 = qt * QT
                po = psA.tile([128, 512], f32, tag="proj", name=f"po{qt}{m}")
                for kc2 in range(2):
                    nc.tensor.matmul(
                        po[:],
                        wo_sb[:, kc2, m * 128:(m + 1) * 128],
                        ctxT_sb[:, kc2, q0:q0 + QT],
                        start=(kc2 == 0), stop=(kc2 == 1),
                    )
                ot = spool.tile([128, 512], f32, tag="wout")
                if tail:
                    nc.scalar.copy(ot[:], po[:])  # ACT is idle in the drain
                else:
                    nc.vector.tensor_copy(ot[:], po[:])
                nc.sync.dma_start(
                    out_d[m * 128:(m + 1) * 128, q0:q0 + QT], ot[:]
                )

            # ---------- schedule ----------
            from collections import deque

            def pk(m, s4):
                return lambda: proj_qk_g(wk_sb, bk_sb, xk_sb, kT_sb, m, s4)

            def pq(m, s4):
                return lambda: proj_qk_g(wq_sb, bq_sb, xq_sb, qT_sb, m, s4)

            def pv_(st):
                return lambda: proj_v_g(st)

            # pair 0 attention (ascending qtiles) pipelines against the
            # remaining projection groups, fed one per attention chunk.
            # Emission order IS program order for Tile: every tile write
            # must be emitted before its first (program-order) reader, so
            # the K/Q m0 projections a qtile depends on are emitted right
            # before it and each V s-tile strictly before the chunk whose
            # AV matmul consumes it.
            fill = deque()
            fill += [pv_(0), pv_(1), pv_(2), pv_(3)]
            fill += [pk(1, 0), pv_(4), pq(1, 0), pv_(5),
                     pk(1, 1), pv_(6), pq(1, 1), pv_(7)]
            noop = lambda: None
            fill += [noop, noop, noop, noop,
                     pv_(8), pv_(9), pv_(10), pv_(11)]
            wo_fill = deque()

            def wof(qt):
                return [(lambda q, m: lambda: wo_g(q, m))(qt, m)
                        for m in range(8)]

            # the pipelined schedule needs qtile qt to touch only kchunks
            # <= 4*qt+3 (true for causal masks); otherwise emit everything
            # up front in dependency-safe order
            max_kc = [
                max((kc for kc in range(NKC)
                     if classes[qt][kc][0] != "skip"), default=-1)
                for qt in range(NQT)
            ]
            pipelined = all(max_kc[qt] <= 4 * qt + 3 for qt in range(NQT))

            if pipelined:
                for qt in range(3):
                    proj_qk_g(wk_sb, bk_sb, xk_sb, kT_sb, 0, qt)
                    proj_qk_g(wq_sb, bq_sb, xq_sb, qT_sb, 0, qt)
                    attn(qt, 0, fill)
                # pair-1 qtiles interleave into the pair-0 stream once the
                # m1 projections (fed through `fill` above) are in; each
                # finished qtile's Wo groups fill later attention blocks
                attn(0, 1, fill)
                wo_fill += wof(0)
                proj_qk_g(wk_sb, bk_sb, xk_sb, kT_sb, 0, 3)
                proj_qk_g(wq_sb, bq_sb, xq_sb, qT_sb, 0, 3)
                while fill:  # m1-proj leftovers must precede attn(*, 1)
                    fill.popleft()()
                wo_fill += [noop, noop, noop, noop,
                            pv_(12), pv_(13), pv_(14), pv_(15)]
                attn(3, 0, wo_fill)
                # late-phase attention is ACT-bound: absorb the remaining m1
                # projections here instead of the PE-bound early phase
                wo_fill += [pk(1, 2), pq(1, 2), pk(1, 3), pq(1, 3)]
                attn(1, 1, wo_fill)
                wo_fill += wof(1)
                attn(2, 1, wo_fill)
                wo_fill += wof(2)
                attn(3, 1, wo_fill)
            else:
                for m in range(2):
                    for s4 in range(NQT):
                        proj_qk_g(wk_sb, bk_sb, xk_sb, kT_sb, m, s4)
                        proj_qk_g(wq_sb, bq_sb, xq_sb, qT_sb, m, s4)
                while fill:
                    fill.popleft()()  # V projections s0-11 and m1 leftovers
                for st in range(12, NKC):
                    proj_v_g(st)
                for qt in range(NQT):
                    attn(qt, 0, wo_fill)
                for qt in range(NQT):
                    attn(qt, 1, wo_fill)
                    wo_fill += wof(qt)
            for j in range(len(wo_fill)):
                wo_fill.popleft()
            for m in range(8):
                wo_g(NQT - 1, m, tail=(m % 2 == 0))

    nc.compile()
    return nc


def _get_nc(classes, has_gen, skip_bias):
    key = (classes, has_gen, skip_bias)
    if key not in _NC_CACHE:
        _NC_CACHE[key] = _build(classes, has_gen, skip_bias)
    return _NC_CACHE[key]


def _xshard(x):  # [S, D] f32 -> [4, 128, 8, 512] bf16 (x^T tiles)
    xt = np.ascontiguousarray(np.asarray(x, F32).T)          # [D, S]
    a = xt.reshape(8, 128, NQT, QT).transpose(2, 1, 0, 3)    # [4, 128, 8, 512]
    return np.ascontiguousarray(a).astype(BF16)


def _wshard(W, g):  # Linear weight [D, D] -> lhsT tiles [128, 8, 256] bf16
    Wt = np.asarray(W, F32).T[:, g * DCH:(g + 1) * DCH]      # [D, 256]
    return np.ascontiguousarray(
        Wt.reshape(8, 128, DCH).transpose(1, 0, 2)
    ).astype(BF16)


def _woshard(W, g):  # Wo [D, D] -> [128, 2, D] bf16 (rows = this core's ch)
    Wt = np.asarray(W, F32).T[g * DCH:(g + 1) * DCH, :]      # [256, D]
    return np.ascontiguousarray(
        Wt.reshape(2, 128, D).transpose(1, 0, 2)
    ).astype(BF16)


def _prep_in_maps(inputs, has_gen):
    pm = np.asarray(inputs["padding_mask"], F32)
    tril_np = np.where(
        np.arange(128)[:, None] <= np.arange(128)[None, :], 0.0, NEG_BIG
    ).astype(BF16)
    ident_np = np.eye(128, dtype=np.float32).astype(BF16)
    maskT = None
    if has_gen:
        # the kernel folds the 1/8 logit scale into exp *after* the mask add,
        # so pre-scale the mask by 8 to compensate
        maskT = np.ascontiguousarray(
            np.asarray(inputs["attention_mask"], F32).T * 8.0
        )

    xs = {n: [_xshard(np.asarray(inputs[n], F32)[b]) for b in range(B)]
          for n in ("q", "k", "v")}
    ws = {n: [_wshard(inputs[w], g) for g in range(GROUPS)]
          for n, w in (("wq", "Wq"), ("wk", "Wk"), ("wv", "Wv"))}
    wos = [_woshard(inputs["Wo"], g) for g in range(GROUPS)]
    bs = {n: np.asarray(inputs[b], F32).reshape(GROUPS, 1, DCH).astype(BF16)
          for n, b in (("bq", "bq"), ("bk", "bk"), ("bv", "bv"))}
    pads = [
        np.ascontiguousarray(pm[b].reshape(NKC, 128).T).astype(F32)
        for b in range(B)
    ]

    in_maps = []
    for c in range(NCORES):
        b, g = divmod(c, GROUPS)
        m = {
            "xq": xs["q"][b], "xk": xs["k"][b], "xv": xs["v"][b],
            "wq": ws["wq"][g], "wk": ws["wk"][g], "wv": ws["wv"][g],
            "wo": wos[g],
            "bq": bs["bq"][g], "bk": bs["bk"][g], "bv": bs["bv"][g],
            "pad": pads[b],
            "tril": tril_np,
            "ident": ident_np,
        }
        if has_gen:
            m["maskT"] = maskT
        in_maps.append(m)
    return in_maps


def _run(inputs, trace=False, **kw):
    mask = np.asarray(inputs["attention_mask"], F32)
    classes, has_gen = _classify(mask)
    skip_bias = not any(
        np.asarray(inputs[b], F32).any() for b in ("bq", "bk", "bv")
    )
    nc = _get_nc(classes, has_gen, skip_bias)
    in_maps = _prep_in_maps(inputs, has_gen)
    try:
        res = run_bass_kernel_spmd(
            nc, in_maps, list(range(NCORES)), trace=trace, **kw
        )
    except (ImportError, ModuleNotFoundError):
        # NTFF profiling hook unavailable in this container
        res = run_bass_kernel_spmd(
            nc, in_maps, list(range(NCORES)), trace=False, **kw
        )
    outs = np.zeros((B, S, D), F32)
    for c in range(NCORES):
        b, _ = divmod(c, GROUPS)
        outs[b] += np.asarray(res.results[c]["out"], F32).T
    outs += np.asarray(inputs["bo"], F32)[None, None, :]
    return outs, res


def kernel(**inputs) -> np.ndarray:
    out, _ = _run(inputs, trace=False)
    return out



# revision 45
# speedup vs baseline: 1.1116x; 1.1116x over previous
"""Trainium2 Bass kernel for nn_MultiHeadAttention (B=2, S=2048, D=1024, H=16).

Sharding: 8 cores = data-parallel over batch (2) x tensor-parallel over heads
(4 groups of 4 heads).  Each core:
  - computes Q^T, K^T (transposed [channels, seq] layout) and V (natural
    [seq, channels] layout) for its 4 heads via bf16 matmuls,
  - runs causal flash attention with transposed logits [k, q]; the AV
    product is computed *flipped* (ctx in [q, d] layout with the exp'd
    attention block as the stationary matmul operand) so each AV matmul
    streams only 64 output columns; softmax denominators come from tiny
    N=1 ones-matmuls accumulated per q-partition,
  - normalizes per q-subtile on DVE (reciprocal broadcast along the free
    dim), PE-transposes ctx back to [ch, q] and multiplies by a
    row-sharded slice of Wo, producing a partial [D, S] output in f32.
Host side: shards/pre-transposes inputs, sums the 4 partial Wo products per
batch entry (the tensor-parallel reduction) and adds the output bias.
"""

import sys

for _p in ("/opt/trn_rl_repo", "/root/.axon_site/_ro/trn_rl_repo"):
    if _p not in sys.path:
        sys.path.insert(0, _p)

import numpy as np
import ml_dtypes

import concourse.bass as bass  # noqa: F401  (registers engines)
import concourse.mybir as mybir
import concourse.tile as tile
from concourse import bacc
from concourse.bass_utils import run_bass_kernel_spmd

BF16 = ml_dtypes.bfloat16
F32 = np.float32

B, S, D, H, HD = 2, 2048, 1024, 16, 64
NCORES = 8
GROUPS = NCORES // B        # 4 head groups
HPG = H // GROUPS           # 4 heads per core
DCH = HPG * HD              # 256 channels per core
QT = 512                    # query tile width
KC = 128                    # key chunk (partition dim)
NQT, NKC = S // QT, S // KC  # 4, 16
NSB = QT // KC              # 4 q-subtiles per qtile
SCALE = 1.0 / 8.0           # 1/sqrt(HD)
NEG_BIG = -1.0e9

_NC_CACHE: dict = {}


def _classify(mask: np.ndarray):
    """Classify each (qtile, kchunk) block of the additive attention mask.

    Returns tuple-of-tuples of (kind, off) with kind in
    {skip, full, diag, gen}; off is the first visible column for diag blocks.
    """
    classes = []
    for qt in range(NQT):
        q0 = qt * QT
        row = []
        for kc in range(NKC):
            k0 = kc * KC
            blk = mask[q0:q0 + QT, k0:k0 + KC]
            if np.all(blk <= -1e8):
                row.append(("skip", 0))
            elif not blk.any():
                row.append(("full", 0))
            else:
                off = k0 - q0
                if 0 <= off < QT:
                    qi = np.arange(q0, q0 + QT)[:, None]
                    ki = np.arange(k0, k0 + KC)[None, :]
                    vis = qi >= ki
                    if (not blk[vis].any()) and np.all(blk[~vis] <= -1e8):
                        row.append(("diag", off))
                        continue
                row.append(("gen", 0))
        classes.append(tuple(row))

    has_gen = any(c[0] == "gen" for r in classes for c in r)
    if has_gen:
        # keep things simple/correct for odd masks: every non-skip block
        # takes the general (full-width + mask add) path
        classes = [
            tuple(("gen", 0) if c[0] in ("diag", "full") else c for c in r)
            for r in classes
        ]
    # first visible chunk of each qtile must cover the full tile width so the
    # accumulating matmul's start=True pass initializes every column
    for r in classes:
        first = next((c for c in r if c[0] != "skip"), None)
        assert first is None or first[1] == 0, "unsupported mask pattern"
    return tuple(tuple(r) for r in classes), has_gen


def _build(classes, has_gen: bool, skip_bias: bool = True, debug: bool = False):
    f32, bf16 = mybir.dt.float32, mybir.dt.bfloat16
    FT = mybir.ActivationFunctionType

    nc = bacc.Bacc("TRN2", target_bir_lowering=False, debug=False)

    # x inputs arrive pre-transposed + bf16, chunked into 4 seq tiles of 512
    xq_d = nc.dram_tensor("xq", [NQT, 128, 8, QT], bf16, kind="ExternalInput")
    xk_d = nc.dram_tensor("xk", [NQT, 128, 8, QT], bf16, kind="ExternalInput")
    xv_d = nc.dram_tensor("xv", [NQT, 128, 8, QT], bf16, kind="ExternalInput")
    wq_d = nc.dram_tensor("wq", [128, 8, DCH], bf16, kind="ExternalInput")
    wk_d = nc.dram_tensor("wk", [128, 8, DCH], bf16, kind="ExternalInput")
    wv_d = nc.dram_tensor("wv", [128, 8, DCH], bf16, kind="ExternalInput")
    wo_d = nc.dram_tensor("wo", [128, 2, D], bf16, kind="ExternalInput")
    bq_d = nc.dram_tensor("bq", [1, DCH], bf16, kind="ExternalInput")
    bk_d = nc.dram_tensor("bk", [1, DCH], bf16, kind="ExternalInput")
    bv_d = nc.dram_tensor("bv", [1, DCH], bf16, kind="ExternalInput")
    pad_d = nc.dram_tensor("pad", [128, NKC], f32, kind="ExternalInput")
    tril01_d = nc.dram_tensor("tril01", [128, 128], bf16, kind="ExternalInput")
    ident_d = nc.dram_tensor("ident", [128, 128], bf16, kind="ExternalInput")
    maskT_d = None
    if has_gen:
        maskT_d = nc.dram_tensor("maskT", [S, S], f32, kind="ExternalInput")
    # bf16 output halves the per-core output DMA traffic; the host-side
    # gather upcasts and sums in f32 (bf16 rounding adds ~0.1% rms here)
    out_d = nc.dram_tensor("out", [D, S], bf16, kind="ExternalOutput")

    # per-(qtile, subtile) first/last accumulating k-chunk, from the mask
    chunks_of = [
        [(kc, classes[qt][kc]) for kc in range(NKC)
         if classes[qt][kc][0] != "skip"]
        for qt in range(NQT)
    ]

    def j_of(cls, off):
        return off // KC if cls == "diag" else 0

    stop_kc = []
    for qt in range(NQT):
        row = []
        for s in range(NSB):
            writers = [kc for kc, (cls, off) in chunks_of[qt]
                       if j_of(cls, off) <= s]
            assert writers, "subtile with no visible chunks"
            row.append(writers[-1])
        stop_kc.append(row)

    with tile.TileContext(nc) as tc:
        with (
            tc.tile_pool(name="cpool", bufs=1) as cpool,
            tc.tile_pool(name="spool", bufs=4) as spool,
            tc.tile_pool(name="apool", bufs=6) as apool,
            tc.tile_pool(name="psA", bufs=1, space="PSUM") as psA,
            tc.tile_pool(name="psQK", bufs=2, space="PSUM") as psQK,
            tc.tile_pool(name="psAV", bufs=2, space="PSUM") as psAV,
            tc.tile_pool(name="psD", bufs=1, space="PSUM") as psD,
        ):
            # ---------- input DMAs, ordered by first consumption ----------
            wk_sb = cpool.tile([128, 8, DCH], bf16)
            nc.sync.dma_start(wk_sb[:, 0:4, :], wk_d[:, 0:4, :])
            nc.sync.dma_start(wk_sb[:, 4:8, :], wk_d[:, 4:8, :])

            def make_x(name):
                return [
                    cpool.tile([128, 8, QT], bf16, name=f"{name}{s4}")
                    for s4 in range(NQT)
                ]

            xk_sb, xq_sb, xv_sb = make_x("xk"), make_x("xq"), make_x("xv")
            # the first chunks land in small pieces, ordered by consumption,
            # so the projection pipeline starts as early as possible
            nc.sync.dma_start(xk_sb[0][:, 0:2, :], xk_d[0, :, 0:2, :])
            nc.sync.dma_start(xk_sb[0][:, 2:4, :], xk_d[0, :, 2:4, :])
            nc.sync.dma_start(xk_sb[0][:, 4:6, :], xk_d[0, :, 4:6, :])
            nc.sync.dma_start(xk_sb[0][:, 6:8, :], xk_d[0, :, 6:8, :])
            wv_sb = cpool.tile([128, 8, DCH], bf16)
            nc.sync.dma_start(wv_sb[:], wv_d[:])
            nc.sync.dma_start(xv_sb[0][:, 0:4, :], xv_d[0, :, 0:4, :])
            nc.sync.dma_start(xv_sb[0][:, 4:8, :], xv_d[0, :, 4:8, :])
            wq_sb = cpool.tile([128, 8, DCH], bf16)
            nc.sync.dma_start(wq_sb[:], wq_d[:])
            nc.sync.dma_start(xq_sb[0][:, 0:4, :], xq_d[0, :, 0:4, :])
            nc.sync.dma_start(xq_sb[0][:, 4:8, :], xq_d[0, :, 4:8, :])
            nc.sync.dma_start(xq_sb[1][:], xq_d[1])
            nc.sync.dma_start(xk_sb[1][:], xk_d[1])
            nc.sync.dma_start(xv_sb[1][:], xv_d[1])
            nc.sync.dma_start(xq_sb[2][:], xq_d[2])
            nc.sync.dma_start(xk_sb[2][:], xk_d[2])
            nc.sync.dma_start(xv_sb[2][:], xv_d[2])
            nc.sync.dma_start(xq_sb[3][:], xq_d[3])
            nc.sync.dma_start(xk_sb[3][:], xk_d[3])
            nc.sync.dma_start(xv_sb[3][:], xv_d[3])
            wo_sb = cpool.tile([128, 2, D], bf16)
            nc.sync.dma_start(wo_sb[:], wo_d[:])

            # ---------- constants (gpsimd SWDGE queue, tiny) ----------
            pad_sb = cpool.tile([128, NKC], f32)
            nc.gpsimd.dma_start(pad_sb[:], pad_d[:])
            tril01_sb = cpool.tile([128, 128], bf16)
            nc.gpsimd.dma_start(tril01_sb[:], tril01_d[:])
            ident_sb = cpool.tile([128, 128], bf16)
            nc.gpsimd.dma_start(ident_sb[:], ident_d[:])
            ones_col = cpool.tile([128, 1], bf16)
            nc.gpsimd.memset(ones_col[:], 1.0)
            ones = cpool.tile([1, 512], bf16)
            nc.gpsimd.memset(ones[:], 1.0)
            bq_sb = cpool.tile([1, DCH], bf16)
            nc.gpsimd.dma_start(bq_sb[:], bq_d[:])
            bk_sb = cpool.tile([1, DCH], bf16)
            nc.gpsimd.dma_start(bk_sb[:], bk_d[:])
            bv_sb = cpool.tile([1, DCH], bf16)
            nc.gpsimd.dma_start(bv_sb[:], bv_d[:])

            qT_sb = cpool.tile([128, 2, S], bf16)   # [pair-stacked ch, pair, s]
            kT_sb = cpool.tile([128, 2, S], bf16)
            v_sb = cpool.tile([128, NKC, HPG, HD], bf16)
            ctxT_sb = cpool.tile([128, 2, S], bf16)
            # softmax denominators; two slots alternate between attention
            # calls so a call's bank-open matmul never waits on the previous
            # call's denominator reads
            dps = psD.tile([128, 2, NSB, 2], f32)

            # ---------- projections ----------
            def proj_qk_g(w_sb, b_sb, x_sb, dst, m, s4, pool=None, tag=None):
                pool = pool or psA
                ps = pool.tile([128, 512], f32, tag=(tag or "proj"),
                               name=f"pp{id(w_sb) % 97}_{m}{s4}")
                for kcc in range(8):
                    nc.tensor.matmul(
                        ps[:],
                        w_sb[:, kcc, m * 128:(m + 1) * 128],
                        x_sb[s4][:, kcc, :],
                        start=(kcc == 0),
                        stop=(kcc == 7 and skip_bias),
                    )
                if not skip_bias:
                    nc.tensor.matmul(
                        ps[:],
                        b_sb[0:1, m * 128:(m + 1) * 128],
                        ones[0:1, 0:512],
                        start=False, stop=True,
                    )
                nc.vector.tensor_copy(
                    dst[:, m, s4 * 512:(s4 + 1) * 512], ps[:]
                )

            def proj_v_g(st, pool=None, tag=None):
                pool = pool or psA
                ps = pool.tile([128, 256], f32, tag=(tag or "proj"),
                               name=f"pv{st}")
                xt = xv_sb[st // 4][:, :, (st % 4) * 128:(st % 4 + 1) * 128]
                for kcc in range(8):
                    nc.tensor.matmul(
                        ps[:],
                        xt[:, kcc, :],
                        wv_sb[:, kcc, :],
                        start=(kcc == 0),
                        stop=(kcc == 7 and skip_bias),
                    )
                if not skip_bias:
                    nc.tensor.matmul(
                        ps[:], ones[0:1, 0:128], bv_sb[0:1, :],
                        start=False, stop=True,
                    )
                # NB: ACT mis-executes this strided 3D copy (keep off ACT)
                nc.vector.tensor_copy(
                    v_sb[:, st, :, :],
                    ps[:].rearrange("p (h d) -> p h d", h=HPG),
                )

            # ---------- attention for one (qtile, pair) ----------
            cs_store: dict = {}
            ncall = [0]

            def attn(qt, p_, filler=None):
                q0 = qt * QT
                chunks = chunks_of[qt]
                slot = ncall[0] % 2
                ncall[0] += 1
                av = psAV.tile([128, NSB, 2, HD], f32, tag="av",
                               name=f"av{qt}_{p_}")
                cs_list = [None] * NSB
                last_kc = chunks[-1][0]
                # PSUM accumulation groups are bank-granular (one open group
                # per 2KB zero region): start only the first matmul into each
                # bank, stop only the last; per-region first-writes overwrite
                # via the pending-zero bytes, later ones accumulate.
                av_first = [True]
                d_first = [True]

                def emit_av(idx, j, kc, att3):
                    last_chunk = kc == last_kc
                    for i in range(2):
                        h_loc = 2 * p_ + i
                        for s in range(j, NSB):
                            last = last_chunk and i == 1 and s == NSB - 1
                            nc.tensor.matmul(
                                av[:, s, i, :],
                                att3[:, i, s * KC:(s + 1) * KC],
                                v_sb[:, kc, h_loc, :],
                                start=av_first[0], stop=last,
                            )
                            av_first[0] = False
                            nc.tensor.matmul(
                                dps[:, slot, s, i:i + 1],
                                att3[:, i, s * KC:(s + 1) * KC],
                                ones_col[:, 0:1],
                                start=d_first[0], stop=last,
                            )
                            d_first[0] = False

                from collections import deque as _dq
                pend = _dq()
                for idx, (kc, (cls, off)) in enumerate(chunks):
                    j = j_of(cls, off)
                    pairps = psQK.tile([128, 1024], f32, tag="qk",
                                       name=f"qk{qt}_{p_}_{kc}")
                    pq = pairps.rearrange("p (h q) -> p h q", h=2)
                    for i in range(2):
                        lo = 64 * i
                        nc.tensor.matmul(
                            pq[:, i, off:QT],
                            kT_sb[lo:lo + 64, p_, kc * KC:(kc + 1) * KC],
                            qT_sb[lo:lo + 64, p_, q0 + off:q0 + QT],
                            start=True, stop=True,
                        )
                    if cls == "gen":
                        mt = spool.tile([128, QT], f32, tag="genmask",
                                        name=f"mt{qt}{p_}{kc}")
                        nc.gpsimd.dma_start(
                            mt[:], maskT_d[kc * KC:(kc + 1) * KC, q0:q0 + QT]
                        )
                        nc.vector.tensor_add(
                            pq[:, :, :], pq[:, :, :],
                            mt[:, None, :].to_broadcast((128, 2, QT)),
                        )
                    at = apool.tile([128, 1024], bf16, tag="attnT")
                    att3 = at.rearrange("p (h q) -> p h q", h=2)
                    nc.scalar.activation(
                        att3[:, :, off:], pq[:, :, off:], FT.Exp,
                        bias=pad_sb[:, kc:kc + 1], scale=SCALE,
                    )
                    if cls == "diag":
                        # zero the not-yet-visible upper triangle of the
                        # boundary block (cheap DVE multiply by a 0/1 mask)
                        nc.vector.tensor_mul(
                            att3[:, :, off:off + KC],
                            att3[:, :, off:off + KC],
                            tril01_sb[:, None, :].to_broadcast((128, 2, KC)),
                        )
                    # weave one group of independent PE work between the exp
                    # and its AV consumers so the PE stream has ready work
                    # while the activation engine computes the exp
                    if filler:
                        filler.popleft()()
                    # software-pipeline by two chunks: AV matmuls trail their
                    # exp so the in-order PE stream never parks on the ACT
                    if len(pend) >= 2:
                        emit_av(*pend.popleft())
                    pend.append((idx, j, kc, att3))
                while pend:
                    emit_av(*pend.popleft())
                # normalize: PSUM reads must wait for the bank group to close
                dsb = spool.tile([128, NSB, 2], f32, tag="dsb", bufs=4,
                                 name=f"dsb{qt}{p_}")
                nc.vector.tensor_copy(dsb[:], dps[:, slot, :, :])
                rcp = spool.tile([128, NSB, 2], f32, tag="rcp", bufs=4,
                                 name=f"rcp{qt}{p_}")
                nc.vector.reciprocal(rcp[:], dsb[:])
                for s in range(NSB):
                    cs = spool.tile([128, 2, HD], bf16, tag="csb", bufs=10,
                                    name=f"cs{qt}{p_}{s}")
                    nc.vector.tensor_mul(
                        cs[:], av[:, s, :, :],
                        rcp[:, s, :, None].to_broadcast((128, 2, HD)),
                    )
                    cs_list[s] = cs
                cs_store[(qt, p_)] = cs_list

            # transpose one (qt, pair)'s scaled ctx back to [ch, q] (filler)
            def transp(qt, p_, fast=False):
                cs_list = cs_store[(qt, p_)]
                psT = psA.tile([128, 512], bf16, tag="proj",
                               name=f"pt{qt}{p_}")
                for s in range(NSB):
                    nc.tensor.transpose(
                        psT[:, s * KC:(s + 1) * KC], cs_list[s][:], ident_sb[:]
                    )
                nc.vector.tensor_copy(
                    ctxT_sb[:, p_, qt * QT:(qt + 1) * QT], psT[:])

            def wo_g(qt, m, pool=None, drain=False):
                q0 = qt * QT
                pool = pool or psA
                tags = {id(psA): "proj", id(psQK): "qk", id(psAV): "av"}
                po = pool.tile([128, 512], f32, tag=tags[id(pool)],
                               name=f"pog{qt}{m}")
                for kc2 in range(2):
                    nc.tensor.matmul(
                        po[:],
                        wo_sb[:, kc2, m * 128:(m + 1) * 128],
                        ctxT_sb[:, kc2, q0:q0 + QT],
                        start=(kc2 == 0), stop=(kc2 == 1),
                    )
                ot = spool.tile([128, 512], bf16, tag="wout", bufs=4,
                                name=f"ot{qt}{m}")
                if drain:  # ACT is idle in the drain; split copies with DVE
                    eng = (nc.scalar.copy, nc.vector.tensor_copy)[m % 2]
                else:
                    eng = nc.vector.tensor_copy
                eng(ot[:], po[:])
                nc.sync.dma_start(
                    out_d[m * 128:(m + 1) * 128, q0:q0 + QT], ot[:]
                )

            # ---------- schedule ----------
            from collections import deque

            def pk(m, s4):
                return lambda: proj_qk_g(wk_sb, bk_sb, xk_sb, kT_sb, m, s4)

            def pq_(m, s4):
                # inline q projections rotate through the psQK pool so they
                # don't serialize with the preceding k projection on psA
                return lambda: proj_qk_g(wq_sb, bq_sb, xq_sb, qT_sb, m, s4,
                                         pool=psQK, tag="qk")

            def pv_(st):
                return lambda: proj_v_g(st)

            def tr(qt, p_):
                return lambda: transp(qt, p_)

            noop = lambda: None

            def wof(qt, m):
                # alternate psA/psQK so consecutive wo fillers never wait on
                # each other's PSUM->SBUF copy
                return lambda: wo_g(qt, m, pool=(psQK if m % 2 else psA))

            # the pipelined schedule needs qtile qt to touch only kchunks
            # <= 4*qt+3 (true for causal masks); otherwise emit everything
            # up front in dependency-safe order
            max_kc = [
                max((kc for kc, _ in chunks_of[qt]), default=-1)
                for qt in range(NQT)
            ]
            pipelined = all(max_kc[qt] <= 4 * qt + 3 for qt in range(NQT))

            if pipelined:
                # prologue: everything that only needs the s4=0 inputs,
                # alternating psA/psQK so groups never wait on each other's
                # PSUM->SBUF copy; all later projections ride the fill
                # stream so the ACT exp cadence is never interrupted by
                # inline projection lumps
                proj_qk_g(wk_sb, bk_sb, xk_sb, kT_sb, 0, 0)
                proj_qk_g(wk_sb, bk_sb, xk_sb, kT_sb, 1, 0, pool=psQK,
                          tag="qk")
                proj_v_g(0)
                proj_v_g(1, pool=psQK, tag="qk")
                proj_qk_g(wq_sb, bq_sb, xq_sb, qT_sb, 0, 0, pool=psQK,
                          tag="qk")
                fill = deque()
                fill += [pv_(2), pq_(0, 1), pv_(3), pq_(1, 0)]
                attn(0, 0, fill)                     # 4 chunks
                fill = deque()
                fill += [pk(0, 1), pv_(4), pq_(0, 2), pv_(5),
                         tr(0, 0), pv_(6), pv_(7), pk(0, 2)]
                attn(1, 0, fill)                     # 8 chunks
                fill = deque()
                fill += [pv_(8), pq_(0, 3), pv_(9), tr(1, 0),
                         pv_(10), pv_(11), pk(0, 3), pk(1, 1),
                         pq_(1, 1), noop, noop, noop]
                attn(2, 0, fill)                     # 12 chunks
                fill = deque()
                fill += [pv_(12), pv_(13), pv_(14), pv_(15)]
                attn(0, 1, fill)                     # 4 chunks
                fill = deque()
                fill += [tr(0, 1), noop]
                fill += [wof(0, m) for m in range(8)]
                fill += [tr(2, 0), noop, noop, noop, noop, noop]
                attn(3, 0, fill)                     # 16 chunks
                fill = deque()
                fill += [pk(1, 2), tr(3, 0), pq_(1, 2), pk(1, 3),
                         pq_(1, 3), noop, noop, noop]
                attn(1, 1, fill)                     # 8 chunks
                fill = deque()
                fill += [noop, tr(1, 1)]
                fill += [wof(1, m) for m in range(8)]
                fill += [noop, noop]
                attn(2, 1, fill)                     # 12 chunks
                fill = deque()
                fill += [noop, tr(2, 1)]
                fill += [wof(2, m) for m in range(8)]
                fill += [noop] * 6
                attn(3, 1, fill)                     # 16 chunks
                transp(3, 1, fast=True)
                pools = (psA, psQK, psAV)
                for m in range(8):
                    wo_g(3, m, pools[m % 3], drain=True)
            else:
                for m in range(2):
                    for s4 in range(NQT):
                        proj_qk_g(wk_sb, bk_sb, xk_sb, kT_sb, m, s4)
                        proj_qk_g(wq_sb, bq_sb, xq_sb, qT_sb, m, s4)
                for st in range(NKC):
                    proj_v_g(st)
                for p_ in range(2):
                    for qt in range(NQT):
                        attn(qt, p_)
                        transp(qt, p_)
                for qt in range(NQT):
                    for m in range(8):
                        wo_g(qt, m, pool=(psQK if m % 2 else psA),
                             drain=True)

    nc.compile()
    return nc


def _get_nc(classes, has_gen, skip_bias):
    key = (classes, has_gen, skip_bias)
    if key not in _NC_CACHE:
        _NC_CACHE[key] = _build(classes, has_gen, skip_bias)
    return _NC_CACHE[key]


def _xshard(x):  # [S, D] f32 -> [4, 128, 8, 512] bf16 (x^T tiles)
    xt = np.ascontiguousarray(np.asarray(x, F32).T)          # [D, S]
    a = xt.reshape(8, 128, NQT, QT).transpose(2, 1, 0, 3)    # [4, 128, 8, 512]
    return np.ascontiguousarray(a).astype(BF16)


def _wshard(W, g):  # Linear weight [D, D] -> lhsT tiles [128, 8, 256] bf16
    Wt = np.asarray(W, F32).T[:, g * DCH:(g + 1) * DCH]      # [D, 256]
    return np.ascontiguousarray(
        Wt.reshape(8, 128, DCH).transpose(1, 0, 2)
    ).astype(BF16)


def _woshard(W, g):  # Wo [D, D] -> [128, 2, D] bf16 (rows = this core's ch)
    Wt = np.asarray(W, F32).T[g * DCH:(g + 1) * DCH, :]      # [256, D]
    return np.ascontiguousarray(
        Wt.reshape(2, 128, D).transpose(1, 0, 2)
    ).astype(BF16)


def _prep_in_maps(inputs, has_gen):
    pm = np.asarray(inputs["padding_mask"], F32)
    tril01_np = np.where(
        np.arange(128)[:, None] <= np.arange(128)[None, :], 1.0, 0.0
    ).astype(BF16)
    ident_np = np.eye(128, dtype=np.float32).astype(BF16)
    maskT = None
    if has_gen:
        # the kernel folds the 1/8 logit scale into exp *after* the mask add,
        # so pre-scale the mask by 8 to compensate
        maskT = np.ascontiguousarray(
            np.asarray(inputs["attention_mask"], F32).T * 8.0
        )

    xs = {n: [_xshard(np.asarray(inputs[n], F32)[b]) for b in range(B)]
          for n in ("q", "k", "v")}
    ws = {n: [_wshard(inputs[w], g) for g in range(GROUPS)]
          for n, w in (("wq", "Wq"), ("wk", "Wk"), ("wv", "Wv"))}
    wos = [_woshard(inputs["Wo"], g) for g in range(GROUPS)]
    bs = {n: np.asarray(inputs[b], F32).reshape(GROUPS, 1, DCH).astype(BF16)
          for n, b in (("bq", "bq"), ("bk", "bk"), ("bv", "bv"))}
    pads = [
        np.ascontiguousarray(pm[b].reshape(NKC, 128).T).astype(F32)
        for b in range(B)
    ]

    in_maps = []
    for c in range(NCORES):
        b, g = divmod(c, GROUPS)
        m = {
            "xq": xs["q"][b], "xk": xs["k"][b], "xv": xs["v"][b],
            "wq": ws["wq"][g], "wk": ws["wk"][g], "wv": ws["wv"][g],
            "wo": wos[g],
            "bq": bs["bq"][g], "bk": bs["bk"][g], "bv": bs["bv"][g],
            "pad": pads[b],
            "tril01": tril01_np,
            "ident": ident_np,
        }
        if has_gen:
            m["maskT"] = maskT
        in_maps.append(m)
    return in_maps


def _run(inputs, trace=False, **kw):
    mask = np.asarray(inputs["attention_mask"], F32)
    classes, has_gen = _classify(mask)
    skip_bias = not any(
        np.asarray(inputs[b], F32).any() for b in ("bq", "bk", "bv")
    )
    nc = _get_nc(classes, has_gen, skip_bias)
    in_maps = _prep_in_maps(inputs, has_gen)
    try:
        res = run_bass_kernel_spmd(
            nc, in_maps, list(range(NCORES)), trace=trace, **kw
        )
    except (ImportError, ModuleNotFoundError):
        # NTFF profiling hook unavailable in this container
        res = run_bass_kernel_spmd(
            nc, in_maps, list(range(NCORES)), trace=False, **kw
        )
    outs = np.zeros((B, S, D), F32)
    for c in range(NCORES):
        b, _ = divmod(c, GROUPS)
        outs[b] += np.asarray(res.results[c]["out"], F32).T
    outs += np.asarray(inputs["bo"], F32)[None, None, :]
    return outs, res


def kernel(**inputs) -> np.ndarray:
    out, _ = _run(inputs, trace=False)
    return out


# revision 57
# speedup vs baseline: 1.1303x; 1.0168x over previous
"""Trainium2 Bass kernel for nn_MultiHeadAttention (B=2, S=2048, D=1024, H=16).

Sharding: 8 cores = data-parallel over batch (2) x tensor-parallel over heads
(4 groups of 4 heads).  Each core:
  - computes Q^T, K^T (transposed [channels, seq] layout) and V (natural
    [seq, channels] layout) for its 4 heads via bf16 matmuls,
  - runs causal flash attention with transposed logits [k, q]; the AV
    product is computed *flipped* (ctx in [q, d] layout with the exp'd
    attention block as the stationary matmul operand) so each AV matmul
    streams only 64 output columns; softmax denominators come from tiny
    N=1 ones-matmuls accumulated per q-partition,
  - normalizes per q-subtile on DVE (reciprocal broadcast along the free
    dim), PE-transposes ctx back to [ch, q] and multiplies by a
    row-sharded slice of Wo, producing a partial [D, S] output in f32.
Host side: shards/pre-transposes inputs, sums the 4 partial Wo products per
batch entry (the tensor-parallel reduction) and adds the output bias.
"""

import sys

for _p in ("/opt/trn_rl_repo", "/root/.axon_site/_ro/trn_rl_repo"):
    if _p not in sys.path:
        sys.path.insert(0, _p)

import numpy as np
import ml_dtypes

import concourse.bass as bass  # noqa: F401  (registers engines)
import concourse.mybir as mybir
import concourse.tile as tile
from concourse import bacc
from concourse.bass_utils import run_bass_kernel_spmd

BF16 = ml_dtypes.bfloat16
F32 = np.float32

B, S, D, H, HD = 2, 2048, 1024, 16, 64
NCORES = 8
GROUPS = NCORES // B        # 4 head groups
HPG = H // GROUPS           # 4 heads per core
DCH = HPG * HD              # 256 channels per core
QT = 512                    # query tile width
KC = 128                    # key chunk (partition dim)
NQT, NKC = S // QT, S // KC  # 4, 16
NSB = QT // KC              # 4 q-subtiles per qtile
SCALE = 1.0 / 8.0           # 1/sqrt(HD)
NEG_BIG = -1.0e9

_NC_CACHE: dict = {}


def _classify(mask: np.ndarray):
    """Classify each (qtile, kchunk) block of the additive attention mask.

    Returns tuple-of-tuples of (kind, off) with kind in
    {skip, full, diag, gen}; off is the first visible column for diag blocks.
    """
    classes = []
    for qt in range(NQT):
        q0 = qt * QT
        row = []
        for kc in range(NKC):
            k0 = kc * KC
            blk = mask[q0:q0 + QT, k0:k0 + KC]
            if np.all(blk <= -1e8):
                row.append(("skip", 0))
            elif not blk.any():
                row.append(("full", 0))
            else:
                off = k0 - q0
                if 0 <= off < QT:
                    qi = np.arange(q0, q0 + QT)[:, None]
                    ki = np.arange(k0, k0 + KC)[None, :]
                    vis = qi >= ki
                    if (not blk[vis].any()) and np.all(blk[~vis] <= -1e8):
                        row.append(("diag", off))
                        continue
                row.append(("gen", 0))
        classes.append(tuple(row))

    has_gen = any(c[0] == "gen" for r in classes for c in r)
    if has_gen:
        # keep things simple/correct for odd masks: every non-skip block
        # takes the general (full-width + mask add) path
        classes = [
            tuple(("gen", 0) if c[0] in ("diag", "full") else c for c in r)
            for r in classes
        ]
    # first visible chunk of each qtile must cover the full tile width so the
    # accumulating matmul's start=True pass initializes every column
    for r in classes:
        first = next((c for c in r if c[0] != "skip"), None)
        assert first is None or first[1] == 0, "unsupported mask pattern"
    return tuple(tuple(r) for r in classes), has_gen


def _build(classes, has_gen: bool, skip_bias: bool = True, debug: bool = False):
    f32, bf16 = mybir.dt.float32, mybir.dt.bfloat16
    FT = mybir.ActivationFunctionType

    nc = bacc.Bacc("TRN2", target_bir_lowering=False, debug=False)

    # x inputs arrive pre-transposed + bf16, chunked into 4 seq tiles of 512
    xq_d = nc.dram_tensor("xq", [NQT, 128, 8, QT], bf16, kind="ExternalInput")
    xk_d = nc.dram_tensor("xk", [NQT, 128, 8, QT], bf16, kind="ExternalInput")
    xv_d = nc.dram_tensor("xv", [NQT, 128, 8, QT], bf16, kind="ExternalInput")
    wq_d = nc.dram_tensor("wq", [128, 8, DCH], bf16, kind="ExternalInput")
    wk_d = nc.dram_tensor("wk", [128, 8, DCH], bf16, kind="ExternalInput")
    wv_d = nc.dram_tensor("wv", [128, 8, DCH], bf16, kind="ExternalInput")
    wo_d = nc.dram_tensor("wo", [128, 2, D], bf16, kind="ExternalInput")
    bq_d = nc.dram_tensor("bq", [1, DCH], bf16, kind="ExternalInput")
    bk_d = nc.dram_tensor("bk", [1, DCH], bf16, kind="ExternalInput")
    bv_d = nc.dram_tensor("bv", [1, DCH], bf16, kind="ExternalInput")
    pad_d = nc.dram_tensor("pad", [128, NKC], f32, kind="ExternalInput")
    tril01_d = nc.dram_tensor("tril01", [128, 128], bf16, kind="ExternalInput")
    ident_d = nc.dram_tensor("ident", [128, 128], bf16, kind="ExternalInput")
    maskT_d = None
    if has_gen:
        maskT_d = nc.dram_tensor("maskT", [S, S], f32, kind="ExternalInput")
    # bf16 output halves the per-core output DMA traffic; the host-side
    # gather upcasts and sums in f32 (bf16 rounding adds ~0.1% rms here)
    out_d = nc.dram_tensor("out", [D, S], bf16, kind="ExternalOutput")

    # per-(qtile, subtile) first/last accumulating k-chunk, from the mask
    chunks_of = [
        [(kc, classes[qt][kc]) for kc in range(NKC)
         if classes[qt][kc][0] != "skip"]
        for qt in range(NQT)
    ]

    def j_of(cls, off):
        return off // KC if cls == "diag" else 0

    stop_kc = []
    for qt in range(NQT):
        row = []
        for s in range(NSB):
            writers = [kc for kc, (cls, off) in chunks_of[qt]
                       if j_of(cls, off) <= s]
            assert writers, "subtile with no visible chunks"
            row.append(writers[-1])
        stop_kc.append(row)

    with tile.TileContext(nc) as tc:
        with (
            tc.tile_pool(name="cpool", bufs=1) as cpool,
            tc.tile_pool(name="spool", bufs=4) as spool,
            tc.tile_pool(name="apool", bufs=8) as apool,
            tc.tile_pool(name="psA", bufs=1, space="PSUM") as psA,
            tc.tile_pool(name="psQK", bufs=2, space="PSUM") as psQK,
            tc.tile_pool(name="psAV", bufs=2, space="PSUM") as psAV,
            tc.tile_pool(name="psD", bufs=1, space="PSUM") as psD,
        ):
            # ---------- input DMAs, ordered by first consumption ----------
            wk_sb = cpool.tile([128, 8, DCH], bf16)
            nc.sync.dma_start(wk_sb[:, 0:4, :], wk_d[:, 0:4, :])
            nc.sync.dma_start(wk_sb[:, 4:8, :], wk_d[:, 4:8, :])

            def make_x(name):
                return [
                    cpool.tile([128, 8, QT], bf16, name=f"{name}{s4}")
                    for s4 in range(NQT)
                ]

            xk_sb, xq_sb, xv_sb = make_x("xk"), make_x("xq"), make_x("xv")
            # the first chunks land in small pieces, ordered by consumption,
            # so the projection pipeline starts as early as possible
            nc.sync.dma_start(xk_sb[0][:, 0:2, :], xk_d[0, :, 0:2, :])
            nc.sync.dma_start(xk_sb[0][:, 2:4, :], xk_d[0, :, 2:4, :])
            nc.sync.dma_start(xk_sb[0][:, 4:6, :], xk_d[0, :, 4:6, :])
            nc.sync.dma_start(xk_sb[0][:, 6:8, :], xk_d[0, :, 6:8, :])
            wv_sb = cpool.tile([128, 8, DCH], bf16)
            nc.sync.dma_start(wv_sb[:], wv_d[:])
            nc.sync.dma_start(xv_sb[0][:, 0:4, :], xv_d[0, :, 0:4, :])
            nc.sync.dma_start(xv_sb[0][:, 4:8, :], xv_d[0, :, 4:8, :])
            wq_sb = cpool.tile([128, 8, DCH], bf16)
            nc.sync.dma_start(wq_sb[:], wq_d[:])
            nc.sync.dma_start(xq_sb[0][:, 0:4, :], xq_d[0, :, 0:4, :])
            nc.sync.dma_start(xq_sb[0][:, 4:8, :], xq_d[0, :, 4:8, :])
            nc.sync.dma_start(xq_sb[1][:], xq_d[1])
            nc.sync.dma_start(xk_sb[1][:], xk_d[1])
            nc.sync.dma_start(xv_sb[1][:], xv_d[1])
            nc.sync.dma_start(xq_sb[2][:], xq_d[2])
            nc.sync.dma_start(xk_sb[2][:], xk_d[2])
            nc.sync.dma_start(xv_sb[2][:], xv_d[2])
            nc.sync.dma_start(xq_sb[3][:], xq_d[3])
            nc.sync.dma_start(xk_sb[3][:], xk_d[3])
            nc.sync.dma_start(xv_sb[3][:], xv_d[3])
            wo_sb = cpool.tile([128, 2, D], bf16)
            nc.sync.dma_start(wo_sb[:], wo_d[:])

            # ---------- constants (gpsimd SWDGE queue, tiny) ----------
            pad_sb = cpool.tile([128, NKC], f32)
            nc.gpsimd.dma_start(pad_sb[:], pad_d[:])
            tril01_sb = cpool.tile([128, 128], bf16)
            nc.gpsimd.dma_start(tril01_sb[:], tril01_d[:])
            ident_sb = cpool.tile([128, 128], bf16)
            nc.gpsimd.dma_start(ident_sb[:], ident_d[:])
            ones_col = cpool.tile([128, 1], bf16)
            nc.gpsimd.memset(ones_col[:], 1.0)
            ones = cpool.tile([1, 512], bf16)
            nc.gpsimd.memset(ones[:], 1.0)
            bq_sb = cpool.tile([1, DCH], bf16)
            nc.gpsimd.dma_start(bq_sb[:], bq_d[:])
            bk_sb = cpool.tile([1, DCH], bf16)
            nc.gpsimd.dma_start(bk_sb[:], bk_d[:])
            bv_sb = cpool.tile([1, DCH], bf16)
            nc.gpsimd.dma_start(bv_sb[:], bv_d[:])

            qT_sb = cpool.tile([128, 2, S], bf16)   # [pair-stacked ch, pair, s]
            kT_sb = cpool.tile([128, 2, S], bf16)
            v_sb = cpool.tile([128, NKC, HPG, HD], bf16)
            ctxT_sb = cpool.tile([128, 2, S], bf16)
            # softmax denominators; two slots alternate between attention
            # calls so a call's bank-open matmul never waits on the previous
            # call's denominator reads
            dps = psD.tile([128, 2, NSB, 2], f32)

            # ---------- projections ----------
            def proj_qk_g(w_sb, b_sb, x_sb, dst, m, s4, pool=None, tag=None):
                pool = pool or psA
                ps = pool.tile([128, 512], f32, tag=(tag or "proj"),
                               name=f"pp{id(w_sb) % 97}_{m}{s4}")
                for kcc in range(8):
                    nc.tensor.matmul(
                        ps[:],
                        w_sb[:, kcc, m * 128:(m + 1) * 128],
                        x_sb[s4][:, kcc, :],
                        start=(kcc == 0),
                        stop=(kcc == 7 and skip_bias),
                    )
                if not skip_bias:
                    nc.tensor.matmul(
                        ps[:],
                        b_sb[0:1, m * 128:(m + 1) * 128],
                        ones[0:1, 0:512],
                        start=False, stop=True,
                    )
                nc.vector.tensor_copy(
                    dst[:, m, s4 * 512:(s4 + 1) * 512], ps[:]
                )

            def proj_v_g(st, pool=None, tag=None):
                pool = pool or psA
                ps = pool.tile([128, 256], f32, tag=(tag or "proj"),
                               name=f"pv{st}")
                xt = xv_sb[st // 4][:, :, (st % 4) * 128:(st % 4 + 1) * 128]
                for kcc in range(8):
                    nc.tensor.matmul(
                        ps[:],
                        xt[:, kcc, :],
                        wv_sb[:, kcc, :],
                        start=(kcc == 0),
                        stop=(kcc == 7 and skip_bias),
                    )
                if not skip_bias:
                    nc.tensor.matmul(
                        ps[:], ones[0:1, 0:128], bv_sb[0:1, :],
                        start=False, stop=True,
                    )
                # NB: ACT mis-executes this strided 3D copy (keep off ACT)
                nc.vector.tensor_copy(
                    v_sb[:, st, :, :],
                    ps[:].rearrange("p (h d) -> p h d", h=HPG),
                )

            # ---------- attention for one (qtile, pair) ----------
            cs_store: dict = {}
            ncall = [0]

            def attn(qt, p_, filler=None):
                q0 = qt * QT
                chunks = chunks_of[qt]
                slot = ncall[0] % 2
                ncall[0] += 1
                av = psAV.tile([128, NSB, 2, HD], f32, tag="av",
                               name=f"av{qt}_{p_}")
                cs_list = [None] * NSB
                last_kc = chunks[-1][0]
                # PSUM accumulation groups are bank-granular (one open group
                # per 2KB zero region): start only the first matmul into each
                # bank, stop only the last; per-region first-writes overwrite
                # via the pending-zero bytes, later ones accumulate.
                av_first = [True]
                d_first = [True]

                def emit_av(idx, j, kc, att3):
                    last_chunk = kc == last_kc
                    for i in range(2):
                        h_loc = 2 * p_ + i
                        for s in range(j, NSB):
                            last = last_chunk and i == 1 and s == NSB - 1
                            nc.tensor.matmul(
                                av[:, s, i, :],
                                att3[:, i, s * KC:(s + 1) * KC],
                                v_sb[:, kc, h_loc, :],
                                start=av_first[0], stop=last,
                            )
                            av_first[0] = False
                            nc.tensor.matmul(
                                dps[:, slot, s, i:i + 1],
                                att3[:, i, s * KC:(s + 1) * KC],
                                ones_col[:, 0:1],
                                start=d_first[0], stop=last,
                            )
                            d_first[0] = False

                from collections import deque as _dq
                pend = _dq()
                for idx, (kc, (cls, off)) in enumerate(chunks):
                    j = j_of(cls, off)
                    pairps = psQK.tile([128, 1024], f32, tag="qk",
                                       name=f"qk{qt}_{p_}_{kc}")
                    pq = pairps.rearrange("p (h q) -> p h q", h=2)
                    for i in range(2):
                        lo = 64 * i
                        nc.tensor.matmul(
                            pq[:, i, off:QT],
                            kT_sb[lo:lo + 64, p_, kc * KC:(kc + 1) * KC],
                            qT_sb[lo:lo + 64, p_, q0 + off:q0 + QT],
                            start=True, stop=True,
                        )
                    if cls == "gen":
                        mt = spool.tile([128, QT], f32, tag="genmask",
                                        name=f"mt{qt}{p_}{kc}")
                        nc.gpsimd.dma_start(
                            mt[:], maskT_d[kc * KC:(kc + 1) * KC, q0:q0 + QT]
                        )
                        nc.vector.tensor_add(
                            pq[:, :, :], pq[:, :, :],
                            mt[:, None, :].to_broadcast((128, 2, QT)),
                        )
                    at = apool.tile([128, 1024], bf16, tag="attnT")
                    att3 = at.rearrange("p (h q) -> p h q", h=2)
                    nc.scalar.activation(
                        att3[:, :, off:], pq[:, :, off:], FT.Exp,
                        bias=pad_sb[:, kc:kc + 1], scale=SCALE,
                    )
                    if cls == "diag":
                        # zero the not-yet-visible upper triangle of the
                        # boundary block (cheap DVE multiply by a 0/1 mask)
                        nc.vector.tensor_mul(
                            att3[:, :, off:off + KC],
                            att3[:, :, off:off + KC],
                            tril01_sb[:, None, :].to_broadcast((128, 2, KC)),
                        )
                    # weave one group of independent PE work between the exp
                    # and its AV consumers so the PE stream has ready work
                    # while the activation engine computes the exp
                    if filler:
                        filler.popleft()()
                    # software-pipeline by two chunks: AV matmuls trail their
                    # exp so the in-order PE stream never parks on the ACT
                    if len(pend) >= 2:
                        emit_av(*pend.popleft())
                    pend.append((idx, j, kc, att3))
                while pend:
                    emit_av(*pend.popleft())
                # normalize: PSUM reads must wait for the bank group to close
                dsb = spool.tile([128, NSB, 2], f32, tag="dsb", bufs=4,
                                 name=f"dsb{qt}{p_}")
                nc.vector.tensor_copy(dsb[:], dps[:, slot, :, :])
                rcp = spool.tile([128, NSB, 2], f32, tag="rcp", bufs=4,
                                 name=f"rcp{qt}{p_}")
                nc.vector.reciprocal(rcp[:], dsb[:])
                for s in range(NSB):
                    cs = spool.tile([128, 2, HD], bf16, tag="csb", bufs=10,
                                    name=f"cs{qt}{p_}{s}")
                    nc.vector.tensor_mul(
                        cs[:], av[:, s, :, :],
                        rcp[:, s, :, None].to_broadcast((128, 2, HD)),
                    )
                    cs_list[s] = cs
                cs_store[(qt, p_)] = cs_list

            # transpose one (qt, pair)'s scaled ctx back to [ch, q] (filler)
            def transp(qt, p_, fast=False):
                cs_list = cs_store[(qt, p_)]
                psT = psA.tile([128, 512], bf16, tag="proj",
                               name=f"pt{qt}{p_}")
                for s in range(NSB):
                    nc.tensor.transpose(
                        psT[:, s * KC:(s + 1) * KC], cs_list[s][:], ident_sb[:]
                    )
                nc.vector.tensor_copy(
                    ctxT_sb[:, p_, qt * QT:(qt + 1) * QT], psT[:])

            def wo_g(qt, m, pool=None, drain=False):
                q0 = qt * QT
                pool = pool or psA
                tags = {id(psA): "proj", id(psQK): "qk", id(psAV): "av"}
                po = pool.tile([128, 512], f32, tag=tags[id(pool)],
                               name=f"pog{qt}{m}")
                for kc2 in range(2):
                    nc.tensor.matmul(
                        po[:],
                        wo_sb[:, kc2, m * 128:(m + 1) * 128],
                        ctxT_sb[:, kc2, q0:q0 + QT],
                        start=(kc2 == 0), stop=(kc2 == 1),
                    )
                ot = spool.tile([128, 512], bf16, tag="wout", bufs=6,
                                name=f"ot{qt}{m}")
                if drain:  # ACT is idle in the drain; split copies with DVE
                    eng = (nc.scalar.copy, nc.vector.tensor_copy)[m % 2]
                else:
                    eng = nc.vector.tensor_copy
                eng(ot[:], po[:])
                nc.sync.dma_start(
                    out_d[m * 128:(m + 1) * 128, q0:q0 + QT], ot[:]
                )

            # ---------- schedule ----------
            from collections import deque

            def pk(m, s4):
                return lambda: proj_qk_g(wk_sb, bk_sb, xk_sb, kT_sb, m, s4)

            def pq_(m, s4):
                # inline q projections rotate through the psQK pool so they
                # don't serialize with the preceding k projection on psA
                return lambda: proj_qk_g(wq_sb, bq_sb, xq_sb, qT_sb, m, s4,
                                         pool=psQK, tag="qk")

            def pv_(st):
                return lambda: proj_v_g(st)

            def tr(qt, p_):
                return lambda: transp(qt, p_)

            noop = lambda: None

            def wof(qt, m):
                # alternate psA/psQK so consecutive wo fillers never wait on
                # each other's PSUM->SBUF copy
                return lambda: wo_g(qt, m, pool=(psQK if m % 2 else psA))

            # the pipelined schedule needs qtile qt to touch only kchunks
            # <= 4*qt+3 (true for causal masks); otherwise emit everything
            # up front in dependency-safe order
            max_kc = [
                max((kc for kc, _ in chunks_of[qt]), default=-1)
                for qt in range(NQT)
            ]
            pipelined = all(max_kc[qt] <= 4 * qt + 3 for qt in range(NQT))

            if pipelined:
                # prologue: everything that only needs the s4=0 inputs,
                # alternating psA/psQK so groups never wait on each other's
                # PSUM->SBUF copy; all later projections ride the fill
                # stream so the ACT exp cadence is never interrupted by
                # inline projection lumps
                proj_qk_g(wk_sb, bk_sb, xk_sb, kT_sb, 0, 0)
                proj_qk_g(wk_sb, bk_sb, xk_sb, kT_sb, 1, 0, pool=psQK,
                          tag="qk")
                proj_v_g(0)
                proj_v_g(1, pool=psQK, tag="qk")
                proj_qk_g(wq_sb, bq_sb, xq_sb, qT_sb, 0, 0, pool=psQK,
                          tag="qk")
                fill = deque()
                fill += [pv_(2), pq_(0, 1), pv_(3), pq_(1, 0)]
                attn(0, 0, fill)                     # 4 chunks
                fill = deque()
                fill += [pk(0, 1), pv_(4), pq_(0, 2), pv_(5),
                         tr(0, 0), pv_(6), pv_(7), pk(0, 2)]
                attn(1, 0, fill)                     # 8 chunks
                fill = deque()
                fill += [pv_(8), pq_(0, 3), pv_(9), tr(1, 0),
                         pv_(10), pv_(11), pk(0, 3), pk(1, 1),
                         pq_(1, 1), noop, noop, noop]
                attn(2, 0, fill)                     # 12 chunks
                fill = deque()
                fill += [pv_(12), pv_(13), pv_(14), pv_(15)]
                attn(0, 1, fill)                     # 4 chunks
                fill = deque()
                fill += [noop, noop, tr(0, 1)]
                fill += [wof(0, m) for m in range(8)]
                fill += [tr(2, 0), noop, noop, noop, noop]
                attn(3, 0, fill)                     # 16 chunks
                fill = deque()
                fill += [pk(1, 2), pq_(1, 2), tr(3, 0),
                         noop, noop, noop, noop, noop]
                attn(1, 1, fill)                     # 8 chunks
                fill = deque()
                fill += [pk(1, 3), pq_(1, 3), noop, tr(1, 1)]
                fill += [wof(1, m) for m in range(7)]
                fill += [noop]
                attn(2, 1, fill)                     # 12 chunks
                fill = deque()
                fill += [wof(1, 7), noop, noop, tr(2, 1)]
                fill += [wof(2, m) for m in range(8)]
                fill += [noop] * 4
                attn(3, 1, fill)                     # 16 chunks
                transp(3, 1, fast=True)
                pools = (psA, psQK, psAV)
                for m in range(8):
                    wo_g(3, m, pools[m % 3], drain=True)
            else:
                for m in range(2):
                    for s4 in range(NQT):
                        proj_qk_g(wk_sb, bk_sb, xk_sb, kT_sb, m, s4)
                        proj_qk_g(wq_sb, bq_sb, xq_sb, qT_sb, m, s4)
                for st in range(NKC):
                    proj_v_g(st)
                for p_ in range(2):
                    for qt in range(NQT):
                        attn(qt, p_)
                        transp(qt, p_)
                for qt in range(NQT):
                    for m in range(8):
                        wo_g(qt, m, pool=(psQK if m % 2 else psA),
                             drain=True)

    nc.compile()
    return nc


def _get_nc(classes, has_gen, skip_bias):
    key = (classes, has_gen, skip_bias)
    if key not in _NC_CACHE:
        _NC_CACHE[key] = _build(classes, has_gen, skip_bias)
    return _NC_CACHE[key]


def _xshard(x):  # [S, D] f32 -> [4, 128, 8, 512] bf16 (x^T tiles)
    xt = np.ascontiguousarray(np.asarray(x, F32).T)          # [D, S]
    a = xt.reshape(8, 128, NQT, QT).transpose(2, 1, 0, 3)    # [4, 128, 8, 512]
    return np.ascontiguousarray(a).astype(BF16)


def _wshard(W, g):  # Linear weight [D, D] -> lhsT tiles [128, 8, 256] bf16
    Wt = np.asarray(W, F32).T[:, g * DCH:(g + 1) * DCH]      # [D, 256]
    return np.ascontiguousarray(
        Wt.reshape(8, 128, DCH).transpose(1, 0, 2)
    ).astype(BF16)


def _woshard(W, g):  # Wo [D, D] -> [128, 2, D] bf16 (rows = this core's ch)
    Wt = np.asarray(W, F32).T[g * DCH:(g + 1) * DCH, :]      # [256, D]
    return np.ascontiguousarray(
        Wt.reshape(2, 128, D).transpose(1, 0, 2)
    ).astype(BF16)


def _prep_in_maps(inputs, has_gen):
    pm = np.asarray(inputs["padding_mask"], F32)
    tril01_np = np.where(
        np.arange(128)[:, None] <= np.arange(128)[None, :], 1.0, 0.0
    ).astype(BF16)
    ident_np = np.eye(128, dtype=np.float32).astype(BF16)
    maskT = None
    if has_gen:
        # the kernel folds the 1/8 logit scale into exp *after* the mask add,
        # so pre-scale the mask by 8 to compensate
        maskT = np.ascontiguousarray(
            np.asarray(inputs["attention_mask"], F32).T * 8.0
        )

    xs = {n: [_xshard(np.asarray(inputs[n], F32)[b]) for b in range(B)]
          for n in ("q", "k", "v")}
    ws = {n: [_wshard(inputs[w], g) for g in range(GROUPS)]
          for n, w in (("wq", "Wq"), ("wk", "Wk"), ("wv", "Wv"))}
    wos = [_woshard(inputs["Wo"], g) for g in range(GROUPS)]
    bs = {n: np.asarray(inputs[b], F32).reshape(GROUPS, 1, DCH).astype(BF16)
          for n, b in (("bq", "bq"), ("bk", "bk"), ("bv", "bv"))}
    pads = [
        np.ascontiguousarray(pm[b].reshape(NKC, 128).T).astype(F32)
        for b in range(B)
    ]

    in_maps = []
    for c in range(NCORES):
        b, g = divmod(c, GROUPS)
        m = {
            "xq": xs["q"][b], "xk": xs["k"][b], "xv": xs["v"][b],
            "wq": ws["wq"][g], "wk": ws["wk"][g], "wv": ws["wv"][g],
            "wo": wos[g],
            "bq": bs["bq"][g], "bk": bs["bk"][g], "bv": bs["bv"][g],
            "pad": pads[b],
            "tril01": tril01_np,
            "ident": ident_np,
        }
        if has_gen:
            m["maskT"] = maskT
        in_maps.append(m)
    return in_maps


def _run(inputs, trace=False, **kw):
    mask = np.asarray(inputs["attention_mask"], F32)
    classes, has_gen = _classify(mask)
    skip_bias = not any(
        np.asarray(inputs[b], F32).any() for b in ("bq", "bk", "bv")
    )
    nc = _get_nc(classes, has_gen, skip_bias)
    in_maps = _prep_in_maps(inputs, has_gen)
    try:
        res = run_bass_kernel_spmd(
            nc, in_maps, list(range(NCORES)), trace=trace, **kw
        )
    except (ImportError, ModuleNotFoundError):
        # NTFF profiling hook unavailable in this container
        res = run_bass_kernel_spmd(
            nc, in_maps, list(range(NCORES)), trace=False, **kw
        )
    outs = np.zeros((B, S, D), F32)
    for c in range(NCORES):
        b, _ = divmod(c, GROUPS)
        outs[b] += np.asarray(res.results[c]["out"], F32).T
    outs += np.asarray(inputs["bo"], F32)[None, None, :]
    return outs, res


def kernel(**inputs) -> np.ndarray:
    out, _ = _run(inputs, trace=False)
    return out


# revision 62
# speedup vs baseline: 1.1408x; 1.0093x over previous
"""Trainium2 Bass kernel for nn_MultiHeadAttention (B=2, S=2048, D=1024, H=16).

Sharding: 8 cores = data-parallel over batch (2) x tensor-parallel over heads
(4 groups of 4 heads).  Each core:
  - computes Q^T, K^T (transposed [channels, seq] layout) and V (natural
    [seq, channels] layout) for its 4 heads via bf16 matmuls,
  - runs causal flash attention with transposed logits [k, q]; the AV
    product is computed *flipped* (ctx in [q, d] layout with the exp'd
    attention block as the stationary matmul operand) so each AV matmul
    streams only 64 output columns; softmax denominators come from tiny
    N=1 ones-matmuls accumulated per q-partition,
  - normalizes per q-subtile on DVE (reciprocal broadcast along the free
    dim), PE-transposes ctx back to [ch, q] and multiplies by a
    row-sharded slice of Wo, producing a partial [D, S] output in f32.
Host side: shards/pre-transposes inputs, sums the 4 partial Wo products per
batch entry (the tensor-parallel reduction) and adds the output bias.
"""

import sys

for _p in ("/opt/trn_rl_repo", "/root/.axon_site/_ro/trn_rl_repo"):
    if _p not in sys.path:
        sys.path.insert(0, _p)

import numpy as np
import ml_dtypes

import concourse.bass as bass  # noqa: F401  (registers engines)
import concourse.mybir as mybir
import concourse.tile as tile
from concourse import bacc
from concourse.bass_utils import run_bass_kernel_spmd

BF16 = ml_dtypes.bfloat16
F32 = np.float32

B, S, D, H, HD = 2, 2048, 1024, 16, 64
NCORES = 8
GROUPS = NCORES // B        # 4 head groups
HPG = H // GROUPS           # 4 heads per core
DCH = HPG * HD              # 256 channels per core
QT = 512                    # query tile width
KC = 128                    # key chunk (partition dim)
NQT, NKC = S // QT, S // KC  # 4, 16
NSB = QT // KC              # 4 q-subtiles per qtile
SCALE = 1.0 / 8.0           # 1/sqrt(HD)
NEG_BIG = -1.0e9

_NC_CACHE: dict = {}


def _classify(mask: np.ndarray):
    """Classify each (qtile, kchunk) block of the additive attention mask.

    Returns tuple-of-tuples of (kind, off) with kind in
    {skip, full, diag, gen}; off is the first visible column for diag blocks.
    """
    classes = []
    for qt in range(NQT):
        q0 = qt * QT
        row = []
        for kc in range(NKC):
            k0 = kc * KC
            blk = mask[q0:q0 + QT, k0:k0 + KC]
            if np.all(blk <= -1e8):
                row.append(("skip", 0))
            elif not blk.any():
                row.append(("full", 0))
            else:
                off = k0 - q0
                if 0 <= off < QT:
                    qi = np.arange(q0, q0 + QT)[:, None]
                    ki = np.arange(k0, k0 + KC)[None, :]
                    vis = qi >= ki
                    if (not blk[vis].any()) and np.all(blk[~vis] <= -1e8):
                        row.append(("diag", off))
                        continue
                row.append(("gen", 0))
        classes.append(tuple(row))

    has_gen = any(c[0] == "gen" for r in classes for c in r)
    if has_gen:
        # keep things simple/correct for odd masks: every non-skip block
        # takes the general (full-width + mask add) path
        classes = [
            tuple(("gen", 0) if c[0] in ("diag", "full") else c for c in r)
            for r in classes
        ]
    # first visible chunk of each qtile must cover the full tile width so the
    # accumulating matmul's start=True pass initializes every column
    for r in classes:
        first = next((c for c in r if c[0] != "skip"), None)
        assert first is None or first[1] == 0, "unsupported mask pattern"
    return tuple(tuple(r) for r in classes), has_gen


def _build(classes, has_gen: bool, skip_bias: bool = True, debug: bool = False):
    f32, bf16 = mybir.dt.float32, mybir.dt.bfloat16
    FT = mybir.ActivationFunctionType

    nc = bacc.Bacc("TRN2", target_bir_lowering=False, debug=False)

    # x inputs arrive pre-transposed + bf16, chunked into 4 seq tiles of 512
    xq_d = nc.dram_tensor("xq", [NQT, 128, 8, QT], bf16, kind="ExternalInput")
    xk_d = nc.dram_tensor("xk", [NQT, 128, 8, QT], bf16, kind="ExternalInput")
    xv_d = nc.dram_tensor("xv", [NQT, 128, 8, QT], bf16, kind="ExternalInput")
    wq_d = nc.dram_tensor("wq", [128, 8, DCH], bf16, kind="ExternalInput")
    wk_d = nc.dram_tensor("wk", [128, 8, DCH], bf16, kind="ExternalInput")
    wv_d = nc.dram_tensor("wv", [128, 8, DCH], bf16, kind="ExternalInput")
    wo_d = nc.dram_tensor("wo", [128, 2, D], bf16, kind="ExternalInput")
    bq_d = nc.dram_tensor("bq", [1, DCH], bf16, kind="ExternalInput")
    bk_d = nc.dram_tensor("bk", [1, DCH], bf16, kind="ExternalInput")
    bv_d = nc.dram_tensor("bv", [1, DCH], bf16, kind="ExternalInput")
    pad_d = nc.dram_tensor("pad", [128, NKC], f32, kind="ExternalInput")
    tril01_d = nc.dram_tensor("tril01", [128, 128], bf16, kind="ExternalInput")
    ident_d = nc.dram_tensor("ident", [128, 128], bf16, kind="ExternalInput")
    maskT_d = None
    if has_gen:
        maskT_d = nc.dram_tensor("maskT", [S, S], f32, kind="ExternalInput")
    # bf16 output halves the per-core output DMA traffic; the host-side
    # gather upcasts and sums in f32 (bf16 rounding adds ~0.1% rms here)
    out_d = nc.dram_tensor("out", [D, S], bf16, kind="ExternalOutput")

    # per-(qtile, subtile) first/last accumulating k-chunk, from the mask
    chunks_of = [
        [(kc, classes[qt][kc]) for kc in range(NKC)
         if classes[qt][kc][0] != "skip"]
        for qt in range(NQT)
    ]

    def j_of(cls, off):
        return off // KC if cls == "diag" else 0

    stop_kc = []
    for qt in range(NQT):
        row = []
        for s in range(NSB):
            writers = [kc for kc, (cls, off) in chunks_of[qt]
                       if j_of(cls, off) <= s]
            assert writers, "subtile with no visible chunks"
            row.append(writers[-1])
        stop_kc.append(row)

    with tile.TileContext(nc) as tc:
        with (
            tc.tile_pool(name="cpool", bufs=1) as cpool,
            tc.tile_pool(name="spool", bufs=4) as spool,
            tc.tile_pool(name="apool", bufs=10) as apool,
            tc.tile_pool(name="psA", bufs=1, space="PSUM") as psA,
            tc.tile_pool(name="psQK", bufs=2, space="PSUM") as psQK,
            tc.tile_pool(name="psAV", bufs=2, space="PSUM") as psAV,
            tc.tile_pool(name="psD", bufs=1, space="PSUM") as psD,
        ):
            # ---------- input DMAs, ordered by first consumption ----------
            wk_sb = cpool.tile([128, 8, DCH], bf16)
            nc.sync.dma_start(wk_sb[:, 0:4, :], wk_d[:, 0:4, :])
            nc.sync.dma_start(wk_sb[:, 4:8, :], wk_d[:, 4:8, :])

            def make_x(name):
                return [
                    cpool.tile([128, 8, QT], bf16, name=f"{name}{s4}")
                    for s4 in range(NQT)
                ]

            xk_sb, xq_sb, xv_sb = make_x("xk"), make_x("xq"), make_x("xv")
            # the first chunks land in small pieces, ordered by consumption,
            # so the projection pipeline starts as early as possible
            nc.sync.dma_start(xk_sb[0][:, 0:2, :], xk_d[0, :, 0:2, :])
            nc.sync.dma_start(xk_sb[0][:, 2:4, :], xk_d[0, :, 2:4, :])
            nc.sync.dma_start(xk_sb[0][:, 4:6, :], xk_d[0, :, 4:6, :])
            nc.sync.dma_start(xk_sb[0][:, 6:8, :], xk_d[0, :, 6:8, :])
            wv_sb = cpool.tile([128, 8, DCH], bf16)
            nc.sync.dma_start(wv_sb[:], wv_d[:])
            nc.sync.dma_start(xv_sb[0][:, 0:4, :], xv_d[0, :, 0:4, :])
            nc.sync.dma_start(xv_sb[0][:, 4:8, :], xv_d[0, :, 4:8, :])
            wq_sb = cpool.tile([128, 8, DCH], bf16)
            nc.sync.dma_start(wq_sb[:], wq_d[:])
            nc.sync.dma_start(xq_sb[0][:, 0:4, :], xq_d[0, :, 0:4, :])
            nc.sync.dma_start(xq_sb[0][:, 4:8, :], xq_d[0, :, 4:8, :])
            nc.sync.dma_start(xq_sb[1][:], xq_d[1])
            nc.sync.dma_start(xk_sb[1][:], xk_d[1])
            nc.sync.dma_start(xv_sb[1][:], xv_d[1])
            nc.sync.dma_start(xq_sb[2][:], xq_d[2])
            nc.sync.dma_start(xk_sb[2][:], xk_d[2])
            nc.sync.dma_start(xv_sb[2][:], xv_d[2])
            nc.sync.dma_start(xq_sb[3][:], xq_d[3])
            nc.sync.dma_start(xk_sb[3][:], xk_d[3])
            nc.sync.dma_start(xv_sb[3][:], xv_d[3])
            wo_sb = cpool.tile([128, 2, D], bf16)
            nc.sync.dma_start(wo_sb[:], wo_d[:])

            # ---------- constants (gpsimd SWDGE queue, tiny) ----------
            pad_sb = cpool.tile([128, NKC], f32)
            nc.gpsimd.dma_start(pad_sb[:], pad_d[:])
            tril01_sb = cpool.tile([128, 128], bf16)
            nc.gpsimd.dma_start(tril01_sb[:], tril01_d[:])
            ident_sb = cpool.tile([128, 128], bf16)
            nc.gpsimd.dma_start(ident_sb[:], ident_d[:])
            ones_col = cpool.tile([128, 1], bf16)
            nc.gpsimd.memset(ones_col[:], 1.0)
            ones = cpool.tile([1, 512], bf16)
            nc.gpsimd.memset(ones[:], 1.0)
            bq_sb = cpool.tile([1, DCH], bf16)
            nc.gpsimd.dma_start(bq_sb[:], bq_d[:])
            bk_sb = cpool.tile([1, DCH], bf16)
            nc.gpsimd.dma_start(bk_sb[:], bk_d[:])
            bv_sb = cpool.tile([1, DCH], bf16)
            nc.gpsimd.dma_start(bv_sb[:], bv_d[:])

            qT_sb = cpool.tile([128, 2, S], bf16)   # [pair-stacked ch, pair, s]
            kT_sb = cpool.tile([128, 2, S], bf16)
            v_sb = cpool.tile([128, NKC, HPG, HD], bf16)
            ctxT_sb = cpool.tile([128, 2, S], bf16)
            # softmax denominators; two slots alternate between attention
            # calls so a call's bank-open matmul never waits on the previous
            # call's denominator reads
            dps = psD.tile([128, 2, NSB, 2], f32)

            # ---------- projections ----------
            def proj_qk_g(w_sb, b_sb, x_sb, dst, m, s4, pool=None, tag=None):
                pool = pool or psA
                ps = pool.tile([128, 512], f32, tag=(tag or "proj"),
                               name=f"pp{id(w_sb) % 97}_{m}{s4}")
                for kcc in range(8):
                    nc.tensor.matmul(
                        ps[:],
                        w_sb[:, kcc, m * 128:(m + 1) * 128],
                        x_sb[s4][:, kcc, :],
                        start=(kcc == 0),
                        stop=(kcc == 7 and skip_bias),
                    )
                if not skip_bias:
                    nc.tensor.matmul(
                        ps[:],
                        b_sb[0:1, m * 128:(m + 1) * 128],
                        ones[0:1, 0:512],
                        start=False, stop=True,
                    )
                nc.vector.tensor_copy(
                    dst[:, m, s4 * 512:(s4 + 1) * 512], ps[:]
                )

            def proj_v_g(st, pool=None, tag=None):
                pool = pool or psA
                ps = pool.tile([128, 256], f32, tag=(tag or "proj"),
                               name=f"pv{st}")
                xt = xv_sb[st // 4][:, :, (st % 4) * 128:(st % 4 + 1) * 128]
                for kcc in range(8):
                    nc.tensor.matmul(
                        ps[:],
                        xt[:, kcc, :],
                        wv_sb[:, kcc, :],
                        start=(kcc == 0),
                        stop=(kcc == 7 and skip_bias),
                    )
                if not skip_bias:
                    nc.tensor.matmul(
                        ps[:], ones[0:1, 0:128], bv_sb[0:1, :],
                        start=False, stop=True,
                    )
                # NB: ACT mis-executes this strided 3D copy (keep off ACT)
                nc.vector.tensor_copy(
                    v_sb[:, st, :, :],
                    ps[:].rearrange("p (h d) -> p h d", h=HPG),
                )

            # ---------- attention for one (qtile, pair) ----------
            cs_store: dict = {}
            ncall = [0]

            def attn(qt, p_, filler=None):
                q0 = qt * QT
                chunks = chunks_of[qt]
                slot = ncall[0] % 2
                ncall[0] += 1
                av = psAV.tile([128, NSB, 2, HD], f32, tag="av",
                               name=f"av{qt}_{p_}")
                cs_list = [None] * NSB
                last_kc = chunks[-1][0]
                # PSUM accumulation groups are bank-granular (one open group
                # per 2KB zero region): start only the first matmul into each
                # bank, stop only the last; per-region first-writes overwrite
                # via the pending-zero bytes, later ones accumulate.
                av_first = [True]
                d_first = [True]

                def emit_av(idx, j, kc, att3):
                    last_chunk = kc == last_kc
                    for i in range(2):
                        h_loc = 2 * p_ + i
                        for s in range(j, NSB):
                            last = last_chunk and i == 1 and s == NSB - 1
                            nc.tensor.matmul(
                                av[:, s, i, :],
                                att3[:, i, s * KC:(s + 1) * KC],
                                v_sb[:, kc, h_loc, :],
                                start=av_first[0], stop=last,
                            )
                            av_first[0] = False
                            nc.tensor.matmul(
                                dps[:, slot, s, i:i + 1],
                                att3[:, i, s * KC:(s + 1) * KC],
                                ones_col[:, 0:1],
                                start=d_first[0], stop=last,
                            )
                            d_first[0] = False

                from collections import deque as _dq
                pend = _dq()
                for idx, (kc, (cls, off)) in enumerate(chunks):
                    j = j_of(cls, off)
                    pairps = psQK.tile([128, 1024], f32, tag="qk",
                                       name=f"qk{qt}_{p_}_{kc}")
                    pq = pairps.rearrange("p (h q) -> p h q", h=2)
                    for i in range(2):
                        lo = 64 * i
                        nc.tensor.matmul(
                            pq[:, i, off:QT],
                            kT_sb[lo:lo + 64, p_, kc * KC:(kc + 1) * KC],
                            qT_sb[lo:lo + 64, p_, q0 + off:q0 + QT],
                            start=True, stop=True,
                        )
                    if cls == "gen":
                        mt = spool.tile([128, QT], f32, tag="genmask",
                                        name=f"mt{qt}{p_}{kc}")
                        nc.gpsimd.dma_start(
                            mt[:], maskT_d[kc * KC:(kc + 1) * KC, q0:q0 + QT]
                        )
                        nc.vector.tensor_add(
                            pq[:, :, :], pq[:, :, :],
                            mt[:, None, :].to_broadcast((128, 2, QT)),
                        )
                    at = apool.tile([128, 1024], bf16, tag="attnT")
                    att3 = at.rearrange("p (h q) -> p h q", h=2)
                    nc.scalar.activation(
                        att3[:, :, off:], pq[:, :, off:], FT.Exp,
                        bias=pad_sb[:, kc:kc + 1], scale=SCALE,
                    )
                    if cls == "diag":
                        # zero the not-yet-visible upper triangle of the
                        # boundary block (cheap DVE multiply by a 0/1 mask)
                        nc.vector.tensor_mul(
                            att3[:, :, off:off + KC],
                            att3[:, :, off:off + KC],
                            tril01_sb[:, None, :].to_broadcast((128, 2, KC)),
                        )
                    # weave one group of independent PE work between the exp
                    # and its AV consumers so the PE stream has ready work
                    # while the activation engine computes the exp
                    if filler:
                        filler.popleft()()
                    # software-pipeline by two chunks: AV matmuls trail their
                    # exp so the in-order PE stream never parks on the ACT
                    if len(pend) >= 2:
                        emit_av(*pend.popleft())
                    pend.append((idx, j, kc, att3))
                while pend:
                    emit_av(*pend.popleft())
                # normalize: PSUM reads must wait for the bank group to close
                rcp = spool.tile([128, NSB, 2], f32, tag="rcp", bufs=4,
                                 name=f"rcp{qt}{p_}")
                nc.vector.reciprocal(rcp[:], dps[:, slot, :, :])
                csq = spool.tile([128, NSB, 2, HD], bf16, tag="csb", bufs=4,
                                 name=f"cs{qt}{p_}")
                nc.vector.tensor_mul(
                    csq[:], av[:, :, :, :],
                    rcp[:, :, :, None].to_broadcast((128, NSB, 2, HD)),
                )
                cs_store[(qt, p_)] = csq

            # transpose one (qt, pair)'s scaled ctx back to [ch, q] (filler)
            def transp(qt, p_, fast=False):
                csq = cs_store[(qt, p_)]
                psT = psA.tile([128, 512], bf16, tag="proj",
                               name=f"pt{qt}{p_}")
                for s in range(NSB):
                    nc.tensor.transpose(
                        psT[:, s * KC:(s + 1) * KC], csq[:, s], ident_sb[:]
                    )
                nc.vector.tensor_copy(
                    ctxT_sb[:, p_, qt * QT:(qt + 1) * QT], psT[:])

            def wo_g(qt, m, pool=None, drain=False):
                q0 = qt * QT
                pool = pool or psA
                tags = {id(psA): "proj", id(psQK): "qk", id(psAV): "av"}
                po = pool.tile([128, 512], f32, tag=tags[id(pool)],
                               name=f"pog{qt}{m}")
                for kc2 in range(2):
                    nc.tensor.matmul(
                        po[:],
                        wo_sb[:, kc2, m * 128:(m + 1) * 128],
                        ctxT_sb[:, kc2, q0:q0 + QT],
                        start=(kc2 == 0), stop=(kc2 == 1),
                    )
                ot = spool.tile([128, 512], bf16, tag="wout", bufs=6,
                                name=f"ot{qt}{m}")
                if drain:  # ACT is idle in the drain; split copies with DVE
                    eng = (nc.scalar.copy, nc.vector.tensor_copy)[m % 2]
                else:
                    eng = nc.vector.tensor_copy
                eng(ot[:], po[:])
                nc.sync.dma_start(
                    out_d[m * 128:(m + 1) * 128, q0:q0 + QT], ot[:]
                )

            # ---------- schedule ----------
            from collections import deque

            def pk(m, s4):
                return lambda: proj_qk_g(wk_sb, bk_sb, xk_sb, kT_sb, m, s4)

            def pq_(m, s4):
                # inline q projections rotate through the psQK pool so they
                # don't serialize with the preceding k projection on psA
                return lambda: proj_qk_g(wq_sb, bq_sb, xq_sb, qT_sb, m, s4,
                                         pool=psQK, tag="qk")

            def pv_(st):
                return lambda: proj_v_g(st)

            def tr(qt, p_):
                return lambda: transp(qt, p_)

            noop = lambda: None

            def wof(qt, m):
                # alternate psA/psQK so consecutive wo fillers never wait on
                # each other's PSUM->SBUF copy
                return lambda: wo_g(qt, m, pool=(psQK if m % 2 else psA))

            # the pipelined schedule needs qtile qt to touch only kchunks
            # <= 4*qt+3 (true for causal masks); otherwise emit everything
            # up front in dependency-safe order
            max_kc = [
                max((kc for kc, _ in chunks_of[qt]), default=-1)
                for qt in range(NQT)
            ]
            pipelined = all(max_kc[qt] <= 4 * qt + 3 for qt in range(NQT))

            if pipelined:
                # prologue: everything that only needs the s4=0 inputs,
                # alternating psA/psQK so groups never wait on each other's
                # PSUM->SBUF copy; all later projections ride the fill
                # stream so the ACT exp cadence is never interrupted by
                # inline projection lumps
                proj_qk_g(wk_sb, bk_sb, xk_sb, kT_sb, 0, 0)
                proj_qk_g(wk_sb, bk_sb, xk_sb, kT_sb, 1, 0, pool=psQK,
                          tag="qk")
                proj_v_g(0)
                proj_v_g(1, pool=psQK, tag="qk")
                proj_qk_g(wq_sb, bq_sb, xq_sb, qT_sb, 0, 0, pool=psQK,
                          tag="qk")
                fill = deque()
                fill += [pv_(2), pq_(0, 1), pv_(3), pq_(1, 0)]
                attn(0, 0, fill)                     # 4 chunks
                fill = deque()
                fill += [pk(0, 1), pv_(4), pq_(0, 2), pv_(5),
                         tr(0, 0), pv_(6), pv_(7), pk(0, 2)]
                attn(1, 0, fill)                     # 8 chunks
                fill = deque()
                fill += [pv_(8), pq_(0, 3), pv_(9), tr(1, 0),
                         pv_(10), pv_(11), pk(0, 3), pk(1, 1),
                         pq_(1, 1), noop, noop, noop]
                attn(2, 0, fill)                     # 12 chunks
                fill = deque()
                fill += [pv_(12), pv_(13), pv_(14), pv_(15)]
                attn(0, 1, fill)                     # 4 chunks
                fill = deque()
                fill += [noop, noop, tr(0, 1)]
                fill += [wof(0, m) for m in range(8)]
                fill += [tr(2, 0), noop, noop, noop, noop]
                attn(3, 0, fill)                     # 16 chunks
                fill = deque()
                fill += [pk(1, 2), pq_(1, 2), tr(3, 0),
                         noop, noop, noop, noop, noop]
                attn(1, 1, fill)                     # 8 chunks
                fill = deque()
                fill += [pk(1, 3), pq_(1, 3), noop, tr(1, 1)]
                fill += [wof(1, m) for m in range(7)]
                fill += [noop]
                attn(2, 1, fill)                     # 12 chunks
                fill = deque()
                fill += [wof(1, 7), noop, noop, tr(2, 1)]
                fill += [wof(2, m) for m in range(8)]
                fill += [noop] * 4
                attn(3, 1, fill)                     # 16 chunks
                transp(3, 1, fast=True)
                pools = (psA, psQK, psAV)
                for m in range(8):
                    wo_g(3, m, pools[m % 3], drain=True)
            else:
                for m in range(2):
                    for s4 in range(NQT):
                        proj_qk_g(wk_sb, bk_sb, xk_sb, kT_sb, m, s4)
                        proj_qk_g(wq_sb, bq_sb, xq_sb, qT_sb, m, s4)
                for st in range(NKC):
                    proj_v_g(st)
                for p_ in range(2):
                    for qt in range(NQT):
                        attn(qt, p_)
                        transp(qt, p_)
                for qt in range(NQT):
                    for m in range(8):
                        wo_g(qt, m, pool=(psQK if m % 2 else psA),
                             drain=True)

    nc.compile()
    return nc


def _get_nc(classes, has_gen, skip_bias):
    key = (classes, has_gen, skip_bias)
    if key not in _NC_CACHE:
        _NC_CACHE[key] = _build(classes, has_gen, skip_bias)
    return _NC_CACHE[key]


def _xshard(x):  # [S, D] f32 -> [4, 128, 8, 512] bf16 (x^T tiles)
    xt = np.ascontiguousarray(np.asarray(x, F32).T)          # [D, S]
    a = xt.reshape(8, 128, NQT, QT).transpose(2, 1, 0, 3)    # [4, 128, 8, 512]
    return np.ascontiguousarray(a).astype(BF16)


def _wshard(W, g):  # Linear weight [D, D] -> lhsT tiles [128, 8, 256] bf16
    Wt = np.asarray(W, F32).T[:, g * DCH:(g + 1) * DCH]      # [D, 256]
    return np.ascontiguousarray(
        Wt.reshape(8, 128, DCH).transpose(1, 0, 2)
    ).astype(BF16)


def _woshard(W, g):  # Wo [D, D] -> [128, 2, D] bf16 (rows = this core's ch)
    Wt = np.asarray(W, F32).T[g * DCH:(g + 1) * DCH, :]      # [256, D]
    return np.ascontiguousarray(
        Wt.reshape(2, 128, D).transpose(1, 0, 2)
    ).astype(BF16)


def _prep_in_maps(inputs, has_gen):
    pm = np.asarray(inputs["padding_mask"], F32)
    tril01_np = np.where(
        np.arange(128)[:, None] <= np.arange(128)[None, :], 1.0, 0.0
    ).astype(BF16)
    ident_np = np.eye(128, dtype=np.float32).astype(BF16)
    maskT = None
    if has_gen:
        # the kernel folds the 1/8 logit scale into exp *after* the mask add,
        # so pre-scale the mask by 8 to compensate
        maskT = np.ascontiguousarray(
            np.asarray(inputs["attention_mask"], F32).T * 8.0
        )

    xs = {n: [_xshard(np.asarray(inputs[n], F32)[b]) for b in range(B)]
          for n in ("q", "k", "v")}
    ws = {n: [_wshard(inputs[w], g) for g in range(GROUPS)]
          for n, w in (("wq", "Wq"), ("wk", "Wk"), ("wv", "Wv"))}
    wos = [_woshard(inputs["Wo"], g) for g in range(GROUPS)]
    bs = {n: np.asarray(inputs[b], F32).reshape(GROUPS, 1, DCH).astype(BF16)
          for n, b in (("bq", "bq"), ("bk", "bk"), ("bv", "bv"))}
    pads = [
        np.ascontiguousarray(pm[b].reshape(NKC, 128).T).astype(F32)
        for b in range(B)
    ]

    in_maps = []
    for c in range(NCORES):
        b, g = divmod(c, GROUPS)
        m = {
            "xq": xs["q"][b], "xk": xs["k"][b], "xv": xs["v"][b],
            "wq": ws["wq"][g], "wk": ws["wk"][g], "wv": ws["wv"][g],
            "wo": wos[g],
            "bq": bs["bq"][g], "bk": bs["bk"][g], "bv": bs["bv"][g],
            "pad": pads[b],
            "tril01": tril01_np,
            "ident": ident_np,
        }
        if has_gen:
            m["maskT"] = maskT
        in_maps.append(m)
    return in_maps


def _run(inputs, trace=False, **kw):
    mask = np.asarray(inputs["attention_mask"], F32)
    classes, has_gen = _classify(mask)
    skip_bias = not any(
        np.asarray(inputs[b], F32).any() for b in ("bq", "bk", "bv")
    )
    nc = _get_nc(classes, has_gen, skip_bias)
    in_maps = _prep_in_maps(inputs, has_gen)
    try:
        res = run_bass_kernel_spmd(
            nc, in_maps, list(range(NCORES)), trace=trace, **kw
        )
    except (ImportError, ModuleNotFoundError):
        # NTFF profiling hook unavailable in this container
        res = run_bass_kernel_spmd(
            nc, in_maps, list(range(NCORES)), trace=False, **kw
        )
    outs = np.zeros((B, S, D), F32)
    for c in range(NCORES):
        b, _ = divmod(c, GROUPS)
        outs[b] += np.asarray(res.results[c]["out"], F32).T
    outs += np.asarray(inputs["bo"], F32)[None, None, :]
    return outs, res


def kernel(**inputs) -> np.ndarray:
    out, _ = _run(inputs, trace=False)
    return out
